# revision 17
# baseline (speedup 1.0000x reference)
"""Trainium2 Bass kernel for nn_ROIHead_TSSEMamba (N=2048 ROIs, 8 cores DP).

Self-contained: host-side packing + Bass/Tile program + SPMD run on 8 cores.

Per-core layout: features on partitions, tokens (roi, t) along free dims.
The selective scan runs on [d_chunk=128, (s=16, roi=128, t=4)] grids with
exp(A*dt) fused into ACT (per-partition scale), one tensor_tensor_scan per
tile, and an in-place tree-add over s.
"""
import numpy as np
from contextlib import ExitStack

import concourse.bass as bass
import concourse.bacc as bacc
import concourse.tile as tile
from concourse import mybir
from concourse.bass_utils import run_bass_kernel_spmd

F16 = mybir.dt.float16
F32 = mybir.dt.float32
AF = mybir.ActivationFunctionType
OP = mybir.AluOpType

NC = 8
NF = 2048
R = NF // NC               # rois per core (256)
L = 4
T = R * L                  # tokens per core (1024)
DM = 512
DI = 1024
S = 16
NL = 2
HID = 4096
HKW = 2                    # roi halves
RH = R // HKW              # rois per half (128)
TH = RH * L                # tokens per half (512)
GRID = S * RH * L          # 8192

_COMPILED = {}


# --------------------------------------------------------------------------
# host-side packing
# --------------------------------------------------------------------------

def _prep(inputs):
    f16 = np.float16
    f32 = np.float32
    g = {}
    wa = np.asarray(inputs['conv_a_w'], f32)            # [256, 512, 3]
    g['wa'] = np.ascontiguousarray(
        wa.transpose(2, 1, 0).reshape(3, 4, 128, 256)).astype(f16)
    wp = np.asarray(inputs['conv_p_w'], f32)[:, :, 0]   # [256, 512]
    g['wp'] = np.ascontiguousarray(wp.T.reshape(4, 128, 256)).astype(f16)
    g['se1'] = np.ascontiguousarray(
        np.asarray(inputs['se_w1'], f32).T.reshape(4, 128, 32) * 0.25).astype(f16)
    g['se2'] = np.ascontiguousarray(np.asarray(inputs['se_w2'], f32).T).astype(f16)
    g['seb1'] = np.asarray(inputs['se_b1'], f32).reshape(32, 1)
    g['seb2'] = np.ascontiguousarray(
        np.asarray(inputs['se_b2'], f32).reshape(4, 128).T)

    lng = np.stack([np.asarray(inputs['ln_g'], f32)[0],
                    np.asarray(inputs['ln_g'], f32)[1],
                    np.asarray(inputs['out_ln_g'], f32)])
    lnb = np.stack([np.asarray(inputs['ln_b'], f32)[0],
                    np.asarray(inputs['ln_b'], f32)[1],
                    np.asarray(inputs['out_ln_b'], f32)])
    g['lng'] = np.ascontiguousarray(
        lng.reshape(3, 4, 128).transpose(2, 0, 1).reshape(128, 12))
    g['lnb'] = np.ascontiguousarray(
        lnb.reshape(3, 4, 128).transpose(2, 0, 1).reshape(128, 12))

    g['inproj'] = np.ascontiguousarray(
        np.asarray(inputs['in_proj_w'], f32).transpose(0, 2, 1)
        .reshape(NL, 4, 128, 16, 128).transpose(0, 1, 3, 2, 4)).astype(f16)

    cw = np.asarray(inputs['conv_w'], f32)              # [2,2,1024,4]
    g['dwcw'] = np.ascontiguousarray(
        cw.reshape(2, 2, 8, 128, 4).transpose(3, 0, 1, 2, 4).reshape(128, 128))
    g['dwcb'] = np.ascontiguousarray(
        np.asarray(inputs['conv_b'], f32).reshape(2, 2, 8, 128)
        .transpose(3, 0, 1, 2).reshape(128, 32))
    g['dtbb'] = np.ascontiguousarray(
        np.asarray(inputs['dt_proj_b'], f32).reshape(2, 2, 8, 128)
        .transpose(3, 0, 1, 2).reshape(128, 32))
    g['dpp'] = np.ascontiguousarray(
        np.asarray(inputs['D_param'], f32).reshape(2, 2, 8, 128)
        .transpose(3, 0, 1, 2).reshape(128, 32))
    A = -np.exp(np.asarray(inputs['A_log'], f32))       # [2,2,1024,16]
    g['app'] = np.ascontiguousarray(
        A.reshape(2, 2, 8, 128, 16).transpose(3, 0, 1, 2, 4).reshape(128, 512))

    g['xpw'] = np.ascontiguousarray(
        np.asarray(inputs['x_proj_w'], f32).transpose(0, 1, 3, 2)
        .reshape(2, 2, 8, 128, 64)).astype(f16)
    g['dtw'] = np.ascontiguousarray(
        np.asarray(inputs['dt_proj_w'], f32).transpose(0, 1, 3, 2)).astype(f16)
    g['ow'] = np.ascontiguousarray(
        np.asarray(inputs['out_proj_w'], f32).transpose(0, 2, 1)
        .reshape(NL, 8, 128, 4, 128).transpose(0, 1, 3, 2, 4)).astype(f16)

    g['w1'] = np.ascontiguousarray(
        np.asarray(inputs['mlp_w1'], f32).T.reshape(4, 128, HID) * 0.25).astype(f16)
    g['b1'] = np.ascontiguousarray(
        np.asarray(inputs['mlp_b1'], f32).reshape(32, 128).T)
    g['w2'] = np.ascontiguousarray(
        np.asarray(inputs['mlp_w2'], f32).T.reshape(32, 128, HID)).astype(f16)
    g['b2'] = np.ascontiguousarray(
        np.asarray(inputs['mlp_b2'], f32).reshape(32, 128).T)
    g['ident'] = np.eye(128, dtype=f32)

    in_maps = []
    xf = np.asarray(inputs['x_flat'], f32)
    for c in range(NC):
        m = dict(g)
        xs = xf[c * R:(c + 1) * R].reshape(R, DM, 7)
        xt = np.transpose(xs, (1, 0, 2))
        xz = np.zeros((DM, R, 9), f32)
        xz[:, :, 1:8] = xt
        xm = np.full((DM, R, 9), -60000.0, f32)
        xm[:, :, 1:8] = xt
        m['xpz'] = np.ascontiguousarray(xz.reshape(4, 128, R * 9)).astype(f16)
        m['xpm'] = np.ascontiguousarray(xm.reshape(4, 128, R * 9)).astype(f16)
        in_maps.append(m)
    return in_maps


# --------------------------------------------------------------------------
# program builder
# --------------------------------------------------------------------------

def _decl(nc):
    d = {}
    def di(name, shape, dt=F16):
        d[name] = nc.dram_tensor(name, shape, dt, kind="ExternalInput").ap()
    di('xpz', [4, 128, R * 9]); di('xpm', [4, 128, R * 9])
    di('wa', [3, 4, 128, 256]); di('wp', [4, 128, 256])
    di('se1', [4, 128, 32]); di('se2', [32, 512])
    di('seb1', [32, 1], F32); di('seb2', [128, 4], F32)
    di('lng', [128, 12], F32); di('lnb', [128, 12], F32)
    di('inproj', [NL, 4, 16, 128, 128])
    di('dwcw', [128, 128], F32); di('dwcb', [128, 32], F32)
    di('dtbb', [128, 32], F32); di('dpp', [128, 32], F32)
    di('app', [128, 512], F32)
    di('xpw', [2, 2, 8, 128, 64]); di('dtw', [2, 2, 32, 1024])
    di('ow', [NL, 8, 4, 128, 128])
    di('w1', [4, 128, HID]); di('b1', [128, 32], F32)
    di('w2', [32, 128, HID]); di('b2', [128, 32], F32)
    di('ident', [128, 128], F32)
    d['out'] = nc.dram_tensor('out', [R, HID], F32, kind="ExternalOutput").ap()
    d['zsp'] = nc.dram_tensor('zsp', [NL, 8, 128, T], F16).ap()
    d['xisp'] = nc.dram_tensor('xisp', [NL, 8, 128, T], F16).ap()
    d['ysum'] = nc.dram_tensor('ysum', [NL, 8, 128, T], F16).ap()
    d['bfd'] = nc.dram_tensor('bfd', [NL, 2, HKW, GRID], F16).ap()
    d['cfd'] = nc.dram_tensor('cfd', [NL, 2, HKW, GRID], F16).ap()
    return d


def ap4(tile_ap, off, dims):
    """AP with partition dim of tile_ap plus given [step,count] free dims."""
    return bass.AP(tensor=tile_ap.tensor, offset=tile_ap.offset + off,
                   ap=[list(tile_ap.ap[0])] + [list(x) for x in dims])


def dram_bcast(dram_ap, nparts, n):
    """Partition-broadcast read AP of a 1-D DRAM region."""
    return bass.AP(tensor=dram_ap.tensor, offset=dram_ap.offset,
                   ap=[[0, nparts], [1, n]])


def build_program(sim_compat=False):
    nc = bacc.Bacc(debug=False)
    d = _decl(nc)
    ctx = ExitStack()
    with ctx:
        tc = ctx.enter_context(tile.TileContext(nc))
        p_const = ctx.enter_context(tc.tile_pool(name="const", bufs=1))
        p_w = ctx.enter_context(tc.tile_pool(name="wts", bufs=3))
        p_tmp = ctx.enter_context(tc.tile_pool(name="tmp", bufs=8))
        p_hres = ctx.enter_context(tc.tile_pool(name="hres", bufs=5))
        ps_a = ctx.enter_context(tc.tile_pool(name="psa", bufs=4, space="PSUM"))
        ps_b = ctx.enter_context(tc.tile_pool(name="psb", bufs=2, space="PSUM"))

        mm = nc.tensor.matmul

        def cload(name, shape, dt=F32):
            t = p_const.tile(shape, dt, tag=name, name=name + "_c")
            nc.sync.dma_start(out=t, in_=d[name])
            return t
        dwcw = cload('dwcw', [128, 128]); dwcb = cload('dwcb', [128, 32])
        dtbb = cload('dtbb', [128, 32]); dpp = cload('dpp', [128, 32])
        app = cload('app', [128, 512])
        lng = cload('lng', [128, 12]); lnb = cload('lnb', [128, 12])
        seb1 = cload('seb1', [32, 1]); seb2 = cload('seb2', [128, 4])
        b1c = cload('b1', [128, 32]); b2c = cload('b2', [128, 32])
        ident = cload('ident', [128, 128])
        ones1 = p_const.tile([128, 1], F16, tag="ones1")
        nc.vector.memset(ones1, 1.0)
        ones_r = p_const.tile([1, 128], F16, tag="ones_r")
        nc.vector.memset(ones_r, 1.0)
        eps_t = p_const.tile([1, 1], F32, tag="eps_t")
        nc.vector.memset(eps_t, 1e-5)
        one32 = p_const.tile([128, 1], F32, tag="one32")
        nc.vector.memset(one32, 1.0)

        def act_silu(out, in_, bias=0.0):
            if not sim_compat:
                nc.scalar.activation(out, in_, AF.Silu, bias=bias)
                return
            sg = p_tmp.tile(list(out.shape), F32, tag="simtmp", bufs=2, name="simsg")
            nc.scalar.activation(sg, in_, AF.Sigmoid, bias=bias)
            pre = p_tmp.tile(list(out.shape), F32, tag="simtmp", bufs=2, name="simpre")
            nc.scalar.activation(pre, in_, AF.Identity, bias=bias)
            nc.vector.tensor_tensor(out=out, in0=sg, in1=pre, op=OP.mult)

        def act_softplus(out, in_, bias):
            # softplus = ln(1 + exp(x)); exp and ln share an ACT table set
            e = p_tmp.tile(list(out.shape), F32, tag="spe", bufs=2, name="sime")
            nc.scalar.activation(e, in_, AF.Exp, bias=bias)
            nc.scalar.activation(out, e, AF.Ln, bias=one32)

        # ==================================================================
        # TSSE
        # ==================================================================
        hres = []
        with tc.tile_pool(name="tsse", bufs=2) as p_ts:
            xpz, xpm = [], []
            for c in range(4):
                tz = p_ts.tile([128, R * 9], F16, tag="xp", bufs=8, name=f"xpz{c}")
                nc.sync.dma_start(out=tz, in_=d['xpz'][c])
                xpz.append(tz)
                tm = p_ts.tile([128, R * 9], F16, tag="xp", bufs=8, name=f"xpm{c}")
                nc.sync.dma_start(out=tm, in_=d['xpm'][c])
                xpm.append(tm)

            y_sb = []
            ps_mt = [ps_b.tile([128, T], F32, tag="psb", name=f"psmt{i}")
                     for i in range(2)]
            first = True
            for k in range(3):
                for kc in range(4):
                    wt = p_w.tile([128, 256], F16, tag="wa", name="wa_t")
                    nc.sync.dma_start(out=wt, in_=d['wa'][k, kc])
                    for mt in range(2):
                        for f in range(2):
                            rhs = ap4(xpz[kc], k + f * 128 * 9, [[9, 128], [2, L]])
                            mm(out=ps_mt[mt][:, f * 512:(f + 1) * 512],
                               lhsT=wt[:, mt * 128:(mt + 1) * 128], rhs=rhs,
                               start=first, stop=(k == 2 and kc == 3))
                    first = False
            for mt in range(2):
                a_t = p_ts.tile([128, T], F16, tag="ya", bufs=4, name=f"ya{mt}")
                nc.scalar.activation(a_t, ps_mt[mt], AF.Relu)
                y_sb.append(a_t)

            p_tiles = []
            for c in range(4):
                m1 = p_ts.tile([128, T], F16, tag="mp", bufs=4, name=f"mp{c}")
                nc.vector.tensor_tensor(out=m1, in0=ap4(xpm[c], 0, [[9, R], [2, L]]),
                                        in1=ap4(xpm[c], 1, [[9, R], [2, L]]),
                                        op=OP.max)
                nc.vector.tensor_tensor(out=m1, in0=m1,
                                        in1=ap4(xpm[c], 2, [[9, R], [2, L]]),
                                        op=OP.max)
                p_tiles.append(m1)
            ps_mt = [ps_b.tile([128, T], F32, tag="psb", name=f"psmu{i}")
                     for i in range(2)]
            for kc in range(4):
                wt = p_w.tile([128, 256], F16, tag="wa", name="wp_t")
                nc.sync.dma_start(out=wt, in_=d['wp'][kc])
                for mt in range(2):
                    for f in range(2):
                        mm(out=ps_mt[mt][:, f * 512:(f + 1) * 512],
                           lhsT=wt[:, mt * 128:(mt + 1) * 128],
                           rhs=p_tiles[kc][:, f * 512:(f + 1) * 512],
                           start=(kc == 0), stop=(kc == 3))
            for mt in range(2):
                p_t = p_ts.tile([128, T], F16, tag="ya", bufs=4, name=f"yb{mt}")
                nc.scalar.activation(p_t, ps_mt[mt], AF.Relu)
                y_sb.append(p_t)

            # SE
            ps1 = ps_a.tile([32, R], F32, tag="psa")
            for kc in range(4):
                ym = p_tmp.tile([128, R], F16, tag="tmp", name="ym")
                with nc.allow_low_precision(reason="4-elem mean"):
                    nc.vector.tensor_reduce(
                        out=ym, in_=y_sb[kc].rearrange("p (r t) -> p r t", t=L),
                        axis=mybir.AxisListType.X, op=OP.add)
                wt = p_w.tile([128, 32], F16, tag="se1", name="se1_t")
                nc.sync.dma_start(out=wt, in_=d['se1'][kc])
                mm(out=ps1, lhsT=wt, rhs=ym, start=(kc == 0), stop=(kc == 3))
            s1 = p_tmp.tile([32, R], F16, tag="tmp", name="s1")
            nc.scalar.activation(s1, ps1, AF.Relu, bias=seb1)
            se2_sb = p_w.tile([32, 512], F16, tag="se2", bufs=1, name="se2_t")
            nc.sync.dma_start(out=se2_sb, in_=d['se2'])
            for c in range(4):
                ps2 = ps_a.tile([128, R], F32, tag="psa")
                mm(out=ps2, lhsT=se2_sb[:, c * 128:(c + 1) * 128], rhs=s1,
                   start=True, stop=True)
                sg = p_tmp.tile([128, R], F16, tag="tmp", name="sg")
                nc.scalar.activation(sg, ps2, AF.Sigmoid, bias=seb2[:, c:c + 1])
                hc = p_hres.tile([128, T], F16, tag="hres", name=f"h0_{c}")
                nc.vector.tensor_tensor(out=hc, in0=y_sb[c],
                                        in1=ap4(sg, 0, [[1, R], [0, L]]), op=OP.mult)
                hres.append(hc)

        # ==================================================================
        # LN helper
        # ==================================================================
        def layer_norm(h_chunks, ln_idx):
            psm = [ps_a.tile([1, 512], F32, tag="psa", name=f"psm{i}")
                   for i in range(2)]
            ps2 = [ps_a.tile([1, 512], F32, tag="psa", name=f"pss{i}")
                   for i in range(2)]
            for c in range(4):
                hh = p_tmp.tile([128, T], F16, tag="tmp", name="hh")
                nc.vector.tensor_tensor(out=hh, in0=h_chunks[c], in1=h_chunks[c],
                                        op=OP.mult)
                for f in range(2):
                    mm(out=psm[f], lhsT=ones1,
                       rhs=h_chunks[c][:, f * 512:(f + 1) * 512],
                       start=(c == 0), stop=(c == 3))
                    mm(out=ps2[f], lhsT=ones1, rhs=hh[:, f * 512:(f + 1) * 512],
                       start=(c == 0), stop=(c == 3))
            rstd = p_tmp.tile([1, T], F16, tag="lnfl", bufs=2, name="rstd")
            mrs = p_tmp.tile([1, T], F16, tag="lnfl", bufs=2, name="mrs")
            for f in range(2):
                mean = p_tmp.tile([1, 512], F32, tag="lnfs", bufs=4, name="mean")
                nc.scalar.mul(mean, psm[f], 1.0 / DM)
                ex2 = p_tmp.tile([1, 512], F32, tag="lnfs", bufs=4, name="ex2")
                nc.scalar.mul(ex2, ps2[f], 1.0 / DM)
                var = p_tmp.tile([1, 512], F32, tag="lnfs", bufs=4, name="var")
                nc.vector.tensor_tensor(out=var, in0=mean, in1=mean, op=OP.mult)
                nc.vector.tensor_tensor(out=var, in0=ex2, in1=var, op=OP.subtract)
                sd = p_tmp.tile([1, 512], F32, tag="lnfs", bufs=4, name="sd")
                nc.scalar.activation(sd, var, AF.Sqrt, bias=eps_t)
                rs32 = p_tmp.tile([1, 512], F32, tag="lnfs", bufs=4, name="rs32")
                nc.vector.reciprocal(rs32, sd)
                fsl = slice(f * 512, (f + 1) * 512)
                nc.vector.tensor_copy(rstd[:, fsl], rs32)
                nc.vector.tensor_tensor(out=mrs[:, fsl], in0=mean, in1=rs32,
                                        op=OP.mult)
            rstd_b = ps_b.tile([128, T], F32, tag="psb", name="rstd_b")
            mrs_b = ps_b.tile([128, T], F32, tag="psb", name="mrs_b")
            for f in range(2):
                fsl = slice(f * 512, (f + 1) * 512)
                mm(out=rstd_b[:, fsl], lhsT=ones_r, rhs=rstd[:, fsl],
                   start=True, stop=True)
                mm(out=mrs_b[:, fsl], lhsT=ones_r, rhs=mrs[:, fsl],
                   start=True, stop=True)
            out_chunks = []
            for c in range(4):
                t1 = p_tmp.tile([128, T], F16, tag="tmp", name="lnt1")
                nc.vector.tensor_tensor(out=t1, in0=h_chunks[c], in1=rstd_b,
                                        op=OP.mult)
                nc.vector.tensor_tensor(out=t1, in0=t1, in1=mrs_b, op=OP.subtract)
                t2 = p_tmp.tile([128, T], F16, tag="hn", bufs=5, name="hn_c")
                col = ln_idx * 4 + c
                nc.vector.tensor_scalar(out=t2, in0=t1,
                                        scalar1=lng[:, col:col + 1],
                                        scalar2=lnb[:, col:col + 1],
                                        op0=OP.mult, op1=OP.add)
                out_chunks.append(t2)
            return out_chunks

        # ==================================================================
        # mamba layers
        # ==================================================================
        with tc.tile_pool(name="xi", bufs=3) as p_xi, \
             tc.tile_pool(name="u", bufs=9) as p_u, \
             tc.tile_pool(name="dth", bufs=2) as p_dth, \
             tc.tile_pool(name="grid", bufs=2) as p_grid, \
             tc.tile_pool(name="bc", bufs=2) as p_bc:
            for l in range(NL):
                hn = layer_norm(hres, l)
                # ---- in_proj ----
                for m in range(16):
                    ps = ps_a.tile([128, 512], F32, tag="psa", name="ipp0")
                    ps2 = ps_a.tile([128, 512], F32, tag="psa", name="ipp1")
                    for kc in range(4):
                        wt = p_w.tile([128, 128], F16, tag="w128", bufs=6, name="ipw")
                        nc.sync.dma_start(out=wt, in_=d['inproj'][l, kc, m])
                        mm(out=ps, lhsT=wt, rhs=hn[kc][:, 0:512],
                           start=(kc == 0), stop=(kc == 3))
                        mm(out=ps2, lhsT=wt, rhs=hn[kc][:, 512:1024],
                           start=(kc == 0), stop=(kc == 3))
                    t = p_tmp.tile([128, T], F16, tag="tmp", name="ipo")
                    if m < 8:
                        nc.scalar.activation(t[:, 0:512], ps, AF.Copy)
                        nc.scalar.activation(t[:, 512:1024], ps2, AF.Copy)
                        nc.sync.dma_start(out=d['xisp'][l, m], in_=t)
                    else:
                        act_silu(t[:, 0:512], ps)
                        act_silu(t[:, 512:1024], ps2)
                        nc.sync.dma_start(out=d['zsp'][l, m - 8], in_=t)

                for dir_ in range(2):
                    ldc = (l * 2 + dir_) * 8
                    # ---- dwconv + silu ----
                    u = []
                    for c in range(8):
                        xic = p_xi.tile([128, T], F16, tag="xi", name="xic")
                        nc.sync.dma_start(out=xic, in_=d['xisp'][l, c])
                        uacc = p_tmp.tile([128, T], F16, tag="tmp", name="uacc")
                        ci = (ldc + c) * 4
                        if dir_ == 0:
                            src3 = xic
                        else:
                            src3 = ap4(xic, 3, [[4, R], [-1, L]])
                        nc.vector.tensor_scalar(out=uacc, in0=src3,
                                                scalar1=dwcw[:, ci + 3:ci + 4],
                                                scalar2=None, op0=OP.mult)
                        for k in (2, 1, 0):
                            sh = 3 - k
                            n_t = L - sh
                            o_ap = ap4(uacc, sh, [[4, R], [1, n_t]])
                            if dir_ == 0:
                                i_ap = ap4(xic, 0, [[4, R], [1, n_t]])
                            else:
                                i_ap = ap4(xic, 3, [[4, R], [-1, n_t]])
                            nc.vector.scalar_tensor_tensor(
                                out=o_ap, in0=i_ap,
                                scalar=dwcw[:, ci + k:ci + k + 1],
                                in1=o_ap, op0=OP.mult, op1=OP.add)
                        ut = p_u.tile([128, T], F16, tag="u", name="ut")
                        act_silu(ut, uacc, bias=dwcb[:, ldc + c:ldc + c + 1])
                        u.append(ut)
                    # ---- x_proj ----
                    dbl = ps_b.tile([128, T], F32, tag="psb", name="dbl")
                    for kc in range(8):
                        wt = p_w.tile([128, 64], F16, tag="xpw", name="xpw_t")
                        nc.sync.dma_start(out=wt, in_=d['xpw'][l, dir_, kc])
                        for f in range(2):
                            mm(out=dbl[0:64, f * 512:(f + 1) * 512],
                               lhsT=wt, rhs=u[kc][:, f * 512:(f + 1) * 512],
                               start=(kc == 0), stop=(kc == 7))
                    dtr = p_tmp.tile([32, T], F16, tag="tmp", name="dtr")
                    nc.scalar.activation(dtr, dbl[0:32, :], AF.Copy)
                    bc32 = p_tmp.tile([32, T], F16, tag="tmp", name="bc32")
                    nc.scalar.activation(bc32, dbl[32:64, :], AF.Copy)
                    bsb, csb = bc32[0:16, :], bc32[16:32, :]
                    dtw_sb = p_w.tile([32, 1024], F16, tag="dtw", bufs=2, name="dtw_t")
                    nc.sync.dma_start(out=dtw_sb, in_=d['dtw'][l, dir_])

                    for hf in range(HKW):
                        hsl = slice(hf * TH, (hf + 1) * TH)
                        nc.sync.dma_start(out=d['bfd'][l, dir_, hf], in_=bsb[:, hsl])
                        nc.sync.dma_start(out=d['cfd'][l, dir_, hf], in_=csb[:, hsl])
                        brep = p_bc.tile([128, GRID], F16, tag="bc", name="brep")
                        crep = p_bc.tile([128, GRID], F16, tag="bc", name="crep")
                        for i in range(4):
                            nc.sync.dma_start(
                                out=brep[i * 32:(i + 1) * 32],
                                in_=dram_bcast(d['bfd'][l, dir_, hf], 32, GRID))
                            nc.sync.dma_start(
                                out=crep[i * 32:(i + 1) * 32],
                                in_=dram_bcast(d['cfd'][l, dir_, hf], 32, GRID))

                        for c in range(8):
                            psd = ps_a.tile([128, 512], F32, tag="psa", name="psd")
                            mm(out=psd, lhsT=dtw_sb[:, c * 128:(c + 1) * 128],
                               rhs=dtr[:, hsl], start=True, stop=True)
                            dth = p_dth.tile([128, TH], F16, tag="dt", bufs=2,
                                             name="dth")
                            act_softplus(dth, psd,
                                         bias=dtbb[:, ldc + c:ldc + c + 1])
                            dtu = p_dth.tile([128, TH], F16, tag="dtu", bufs=2,
                                             name="dtu")
                            nc.vector.tensor_tensor(out=dtu, in0=dth,
                                                    in1=u[c][:, hsl], op=OP.mult)
                            dA = p_grid.tile([128, GRID], F16, tag="dA", name="dA")
                            (nc.gpsimd if c % 2 else nc.vector).memset(
                                ap4(dA, 0, [[4, S * RH]]), 0.0)
                            in_ap = ap4(dth, 1, [[4, RH], [1, 3]])
                            for s in range(S):
                                o_ap = ap4(dA, s * TH + 1, [[4, RH], [1, 3]])
                                csa = (ldc + c) * S + s
                                nc.scalar.activation(o_ap, in_ap, AF.Exp,
                                                     scale=app[:, csa:csa + 1])
                            w_t = p_grid.tile([128, GRID], F16, tag="wh", name="w_t")
                            nc.gpsimd.tensor_tensor(
                                out=w_t, in0=ap4(dtu, 0, [[0, S], [4, RH], [1, L]]),
                                in1=brep, op=OP.mult)
                            h_t = p_grid.tile([128, GRID], F16, tag="wh", name="h_t")
                            nc.vector.tensor_tensor_scan(out=h_t, data0=dA,
                                                         data1=w_t, initial=0.0,
                                                         op0=OP.mult, op1=OP.add)
                            nc.gpsimd.tensor_tensor(out=h_t, in0=h_t, in1=crep,
                                                    op=OP.mult)
                            nc.vector.tensor_tensor(out=h_t[:, :GRID // 2],
                                                    in0=h_t[:, :GRID // 2],
                                                    in1=h_t[:, GRID // 2:],
                                                    op=OP.add)
                            nc.vector.tensor_tensor(out=h_t[:, :GRID // 4],
                                                    in0=h_t[:, :GRID // 4],
                                                    in1=h_t[:, GRID // 4:GRID // 2],
                                                    op=OP.add)
                            nc.gpsimd.tensor_tensor(out=h_t[:, :GRID // 8],
                                                    in0=h_t[:, :GRID // 8],
                                                    in1=h_t[:, GRID // 8:GRID // 4],
                                                    op=OP.add)
                            nc.vector.tensor_tensor(out=h_t[:, :TH],
                                                    in0=h_t[:, :TH],
                                                    in1=h_t[:, TH:GRID // 8],
                                                    op=OP.add)
                            yd = p_dth.tile([128, TH], F16, tag="yd", bufs=3,
                                            name="yd")
                            if dir_ == 0:
                                nc.vector.scalar_tensor_tensor(
                                    out=yd, in0=u[c][:, hsl],
                                    scalar=dpp[:, ldc + c:ldc + c + 1],
                                    in1=h_t[:, :TH], op0=OP.mult, op1=OP.add)
                                nc.gpsimd.dma_start(out=d['ysum'][l, c][:, hsl],
                                                    in_=yd)
                            else:
                                # write yd already t-flipped (flipped reads)
                                nc.vector.scalar_tensor_tensor(
                                    out=yd,
                                    in0=ap4(u[c], hf * TH + 3, [[4, RH], [-1, L]]),
                                    scalar=dpp[:, ldc + c:ldc + c + 1],
                                    in1=ap4(h_t, 3, [[4, RH], [-1, L]]),
                                    op0=OP.mult, op1=OP.add)
                                nc.gpsimd.dma_start(out=d['ysum'][l, c][:, hsl],
                                                    in_=yd, accum_op=OP.add)

                # ---- gate + out_proj + residual ----
                yg = []
                for c in range(8):
                    sz = p_tmp.tile([128, T], F16, tag="tmp", name="szr")
                    nc.sync.dma_start(out=sz, in_=d['zsp'][l, c])
                    ys = p_tmp.tile([128, T], F16, tag="tmp", name="ysr")
                    nc.sync.dma_start(out=ys, in_=d['ysum'][l, c])
                    t = p_u.tile([128, T], F16, tag="u", name="ygt")
                    nc.vector.tensor_tensor(out=t, in0=ys, in1=sz, op=OP.mult)
                    yg.append(t)
                new_hres = []
                for m in range(4):
                    ps = ps_a.tile([128, 512], F32, tag="psa", name="opp0")
                    ps2 = ps_a.tile([128, 512], F32, tag="psa", name="opp1")
                    for kc in range(8):
                        wt = p_w.tile([128, 128], F16, tag="w128", bufs=6, name="oww")
                        nc.sync.dma_start(out=wt, in_=d['ow'][l, kc, m])
                        mm(out=ps, lhsT=wt, rhs=yg[kc][:, 0:512],
                           start=(kc == 0), stop=(kc == 7))
                        mm(out=ps2, lhsT=wt, rhs=yg[kc][:, 512:1024],
                           start=(kc == 0), stop=(kc == 7))
                    hc = p_hres.tile([128, T], F16, tag="hres", name="hres_n")
                    nc.vector.tensor_tensor(out=hc[:, 0:512], in0=hres[m][:, 0:512],
                                            in1=ps, op=OP.add)
                    nc.vector.tensor_tensor(out=hc[:, 512:1024],
                                            in0=hres[m][:, 512:1024], in1=ps2,
                                            op=OP.add)
                    new_hres.append(hc)
                hres = new_hres

        # ==================================================================
        # out LN + mean pool + MLP
        # ==================================================================
        hn = layer_norm(hres, 2)
        with tc.tile_pool(name="mlp", bufs=6) as p_mlp, \
             tc.tile_pool(name="z1p", bufs=33) as p_z1:
            feat = []
            for c in range(4):
                t = p_mlp.tile([128, R], F16, tag="feat", name="feat_c")
                with nc.allow_low_precision(reason="4-elem mean"):
                    nc.vector.tensor_reduce(
                        out=t, in_=hn[c].rearrange("p (r t) -> p r t", t=L),
                        axis=mybir.AxisListType.X, op=OP.add)
                feat.append(t)
            z1 = []
            for m in range(32):
                ps = ps_a.tile([128, R], F32, tag="psa", name="z1ps")
                for kc in range(4):
                    wt = p_w.tile([128, 128], F16, tag="w128", bufs=6, name="w1t")
                    nc.sync.dma_start(out=wt,
                                      in_=d['w1'][kc, :, m * 128:(m + 1) * 128])
                    mm(out=ps, lhsT=wt, rhs=feat[kc],
                       start=(kc == 0), stop=(kc == 3))
                t = p_z1.tile([128, R], F16, tag="z1", name="z1t")
                nc.scalar.activation(t, ps, AF.Relu, bias=b1c[:, m:m + 1])
                z1.append(t)

            for q in range(4):
                acc_a = [ps_a.tile([128, 512], F32, tag="psa", name=f"acca{i}")
                         for i in range(4)]
                acc_b = [ps_b.tile([128, T], F32, tag="psb", name=f"accb{i}")
                         for i in range(2)]

                def acc_ap(mi):
                    if mi < 4:
                        return acc_a[mi][:, 0:256]
                    j = mi - 4
                    return acc_b[j // 2][:, (j % 2) * 512:(j % 2) * 512 + 256]

                for kc in range(32):
                    slab = p_mlp.tile([128, 1024], F16, tag="w2s", name="slab")
                    nc.sync.dma_start(out=slab,
                                      in_=d['w2'][kc, :, q * 1024:(q + 1) * 1024])
                    for mi in range(8):
                        mm(out=acc_ap(mi), lhsT=slab[:, mi * 128:(mi + 1) * 128],
                           rhs=z1[kc], start=(kc == 0), stop=(kc == 31))
                for mi in range(8):
                    mt = q * 8 + mi
                    o_sb = p_mlp.tile([128, R], F32, tag="osb", name="o_sb")
                    nc.scalar.activation(o_sb, acc_ap(mi),
                                         AF.Relu, bias=b2c[:, mt:mt + 1])
                    for rh in range(2):
                        pst = ps_a.tile([128, 128], F32, tag="psa", name="pst")
                        nc.tensor.transpose(pst, o_sb[:, rh * 128:(rh + 1) * 128],
                                            ident)
                        ot = p_mlp.tile([128, 128], F32, tag="ot", name="ot")
                        nc.vector.tensor_copy(ot, pst)
                        nc.sync.dma_start(
                            out=d['out'][rh * 128:(rh + 1) * 128,
                                         mt * 128:(mt + 1) * 128],
                            in_=ot)
    nc.compile()
    return nc


# --------------------------------------------------------------------------
# entry point
# --------------------------------------------------------------------------

def kernel(**inputs):
    if 'nc' not in _COMPILED:
        _COMPILED['nc'] = build_program()
    nc = _COMPILED['nc']
    in_maps = _prep(inputs)
    res = run_bass_kernel_spmd(nc, in_maps, core_ids=list(range(NC)))
    out = np.concatenate([res.results[c]['out'] for c in range(NC)], axis=0)
    return out.astype(np.float32)


if __name__ == '__main__':
    build_program()
    print("program built ok")


# revision 23
# speedup vs baseline: 1.4378x; 1.4378x over previous
"""Trainium2 Bass kernel for nn_ROIHead_TSSEMamba (N=2048 ROIs, 8 cores DP).

Self-contained: host-side packing + Bass/Tile program + SPMD run on 8 cores.

Per-core layout: features on partitions, tokens (roi, t) along free dims.
The selective scan runs on [d_chunk=128, (s=16, roi=128, t=4)] grids with
exp(A*dt) fused into ACT (per-partition scale), one tensor_tensor_scan per
tile, and an in-place tree-add over s.
"""
import numpy as np
from contextlib import ExitStack

import concourse.bass as bass
import concourse.bacc as bacc
import concourse.tile as tile
from concourse import mybir
from concourse.bass_utils import run_bass_kernel_spmd

F16 = mybir.dt.float16
F32 = mybir.dt.float32
AF = mybir.ActivationFunctionType
OP = mybir.AluOpType

NC = 8
NF = 2048
R = NF // NC               # rois per core (256)
L = 4
T = R * L                  # tokens per core (1024)
DM = 512
DI = 1024
S = 16
NL = 2
HID = 4096
HKW = 2                    # roi halves
RH = R // HKW              # rois per half (128)
TH = RH * L                # tokens per half (512)
GRID = S * RH * L          # 8192

_COMPILED = {}


# --------------------------------------------------------------------------
# host-side packing
# --------------------------------------------------------------------------

def _prep(inputs):
    f16 = np.float16
    f32 = np.float32
    g = {}
    wa = np.asarray(inputs['conv_a_w'], f32)            # [256, 512, 3]
    g['wa'] = np.ascontiguousarray(
        wa.transpose(2, 1, 0).reshape(3, 4, 128, 256)).astype(f16)
    wp = np.asarray(inputs['conv_p_w'], f32)[:, :, 0]   # [256, 512]
    g['wp'] = np.ascontiguousarray(wp.T.reshape(4, 128, 256)).astype(f16)
    g['se1'] = np.ascontiguousarray(
        np.asarray(inputs['se_w1'], f32).T.reshape(4, 128, 32) * 0.25).astype(f16)
    g['se2'] = np.ascontiguousarray(np.asarray(inputs['se_w2'], f32).T).astype(f16)
    g['seb1'] = np.asarray(inputs['se_b1'], f32).reshape(32, 1)
    g['seb2'] = np.ascontiguousarray(
        np.asarray(inputs['se_b2'], f32).reshape(4, 128).T)

    lng = np.stack([np.asarray(inputs['ln_g'], f32)[0],
                    np.asarray(inputs['ln_g'], f32)[1],
                    np.asarray(inputs['out_ln_g'], f32)])
    lnb = np.stack([np.asarray(inputs['ln_b'], f32)[0],
                    np.asarray(inputs['ln_b'], f32)[1],
                    np.asarray(inputs['out_ln_b'], f32)])
    g['lng'] = np.ascontiguousarray(
        lng.reshape(3, 4, 128).transpose(2, 0, 1).reshape(128, 12))
    g['lnb'] = np.ascontiguousarray(
        lnb.reshape(3, 4, 128).transpose(2, 0, 1).reshape(128, 12))

    g['inproj'] = np.ascontiguousarray(
        np.asarray(inputs['in_proj_w'], f32).transpose(0, 2, 1)
        .reshape(NL, 4, 128, 16, 128).transpose(0, 1, 3, 2, 4)).astype(f16)

    cw = np.asarray(inputs['conv_w'], f32)              # [2,2,1024,4]
    g['dwcw'] = np.ascontiguousarray(
        cw.reshape(2, 2, 8, 128, 4).transpose(3, 0, 1, 2, 4).reshape(128, 128))
    g['dwcb'] = np.ascontiguousarray(
        np.asarray(inputs['conv_b'], f32).reshape(2, 2, 8, 128)
        .transpose(3, 0, 1, 2).reshape(128, 32))
    g['dtbb'] = np.ascontiguousarray(
        np.asarray(inputs['dt_proj_b'], f32).reshape(2, 2, 8, 128)
        .transpose(3, 0, 1, 2).reshape(128, 32))
    g['dpp'] = np.ascontiguousarray(
        np.asarray(inputs['D_param'], f32).reshape(2, 2, 8, 128)
        .transpose(3, 0, 1, 2).reshape(128, 32))
    A = -np.exp(np.asarray(inputs['A_log'], f32))       # [2,2,1024,16]
    g['app'] = np.ascontiguousarray(
        A.reshape(2, 2, 8, 128, 16).transpose(3, 0, 1, 2, 4).reshape(128, 512))

    g['xpw'] = np.ascontiguousarray(
        np.asarray(inputs['x_proj_w'], f32).transpose(0, 1, 3, 2)
        .reshape(2, 2, 8, 128, 64)).astype(f16)
    g['dtw'] = np.ascontiguousarray(
        np.asarray(inputs['dt_proj_w'], f32).transpose(0, 1, 3, 2)).astype(f16)
    g['ow'] = np.ascontiguousarray(
        np.asarray(inputs['out_proj_w'], f32).transpose(0, 2, 1)
        .reshape(NL, 8, 128, 4, 128).transpose(0, 1, 3, 2, 4)).astype(f16)

    g['w1'] = np.ascontiguousarray(
        np.asarray(inputs['mlp_w1'], f32).T.reshape(4, 128, HID) * 0.25).astype(f16)
    g['b1'] = np.ascontiguousarray(
        np.asarray(inputs['mlp_b1'], f32).reshape(32, 128).T)
    g['w2'] = np.ascontiguousarray(
        np.asarray(inputs['mlp_w2'], f32).T.reshape(32, 128, HID)).astype(f16)
    g['b2'] = np.ascontiguousarray(
        np.asarray(inputs['mlp_b2'], f32).reshape(32, 128).T)
    g['ident'] = np.eye(128, dtype=f32)

    in_maps = []
    xf = np.asarray(inputs['x_flat'], f32)
    for c in range(NC):
        m = dict(g)
        xs = xf[c * R:(c + 1) * R].reshape(R, DM, 7)
        xt = np.transpose(xs, (1, 0, 2))
        xz = np.zeros((DM, R, 9), f32)
        xz[:, :, 1:8] = xt
        xm = np.full((DM, R, 9), -60000.0, f32)
        xm[:, :, 1:8] = xt
        m['xpz'] = np.ascontiguousarray(xz.reshape(4, 128, R * 9)).astype(f16)
        m['xpm'] = np.ascontiguousarray(xm.reshape(4, 128, R * 9)).astype(f16)
        in_maps.append(m)
    return in_maps


# --------------------------------------------------------------------------
# program builder
# --------------------------------------------------------------------------

def _decl(nc):
    d = {}
    def di(name, shape, dt=F16):
        d[name] = nc.dram_tensor(name, shape, dt, kind="ExternalInput").ap()
    di('xpz', [4, 128, R * 9]); di('xpm', [4, 128, R * 9])
    di('wa', [3, 4, 128, 256]); di('wp', [4, 128, 256])
    di('se1', [4, 128, 32]); di('se2', [32, 512])
    di('seb1', [32, 1], F32); di('seb2', [128, 4], F32)
    di('lng', [128, 12], F32); di('lnb', [128, 12], F32)
    di('inproj', [NL, 4, 16, 128, 128])
    di('dwcw', [128, 128], F32); di('dwcb', [128, 32], F32)
    di('dtbb', [128, 32], F32); di('dpp', [128, 32], F32)
    di('app', [128, 512], F32)
    di('xpw', [2, 2, 8, 128, 64]); di('dtw', [2, 2, 32, 1024])
    di('ow', [NL, 8, 4, 128, 128])
    di('w1', [4, 128, HID]); di('b1', [128, 32], F32)
    di('w2', [32, 128, HID]); di('b2', [128, 32], F32)
    di('ident', [128, 128], F32)
    d['out'] = nc.dram_tensor('out', [R, HID], F32, kind="ExternalOutput").ap()
    d['zsp'] = nc.dram_tensor('zsp', [NL, 8, 128, T], F16).ap()
    d['xisp'] = nc.dram_tensor('xisp', [NL, 8, 128, T], F16).ap()
    d['ysum'] = nc.dram_tensor('ysum', [NL, 8, 128, T], F16).ap()
    d['bfd'] = nc.dram_tensor('bfd', [NL, 2, HKW, GRID], F16).ap()
    d['cfd'] = nc.dram_tensor('cfd', [NL, 2, HKW, GRID], F16).ap()
    return d


def ap4(tile_ap, off, dims):
    """AP with partition dim of tile_ap plus given [step,count] free dims."""
    return bass.AP(tensor=tile_ap.tensor, offset=tile_ap.offset + off,
                   ap=[list(tile_ap.ap[0])] + [list(x) for x in dims])


def dram_bcast(dram_ap, nparts, n):
    """Partition-broadcast read AP of a 1-D DRAM region."""
    return bass.AP(tensor=dram_ap.tensor, offset=dram_ap.offset,
                   ap=[[0, nparts], [1, n]])


def build_program(sim_compat=False):
    nc = bacc.Bacc(debug=False)
    d = _decl(nc)
    ctx = ExitStack()
    with ctx:
        tc = ctx.enter_context(tile.TileContext(nc))
        p_const = ctx.enter_context(tc.tile_pool(name="const", bufs=1))
        p_w = ctx.enter_context(tc.tile_pool(name="wts", bufs=3))
        p_tmp = ctx.enter_context(tc.tile_pool(name="tmp", bufs=7))
        p_hres = ctx.enter_context(tc.tile_pool(name="hres", bufs=5))
        ps_a = ctx.enter_context(tc.tile_pool(name="psa", bufs=4, space="PSUM"))
        ps_b = ctx.enter_context(tc.tile_pool(name="psb", bufs=2, space="PSUM"))

        mm = nc.tensor.matmul

        def cload(name, shape, dt=F32):
            t = p_const.tile(shape, dt, tag=name, name=name + "_c")
            nc.sync.dma_start(out=t, in_=d[name])
            return t
        dwcw = cload('dwcw', [128, 128]); dwcb = cload('dwcb', [128, 32])
        dtbb = cload('dtbb', [128, 32]); dpp = cload('dpp', [128, 32])
        app = cload('app', [128, 512])
        lng = cload('lng', [128, 12]); lnb = cload('lnb', [128, 12])
        seb1 = cload('seb1', [32, 1]); seb2 = cload('seb2', [128, 4])
        b1c = cload('b1', [128, 32]); b2c = cload('b2', [128, 32])
        ident = cload('ident', [128, 128])
        ones1 = p_const.tile([128, 1], F16, tag="ones1")
        nc.vector.memset(ones1, 1.0)
        ones_r = p_const.tile([1, 128], F16, tag="ones_r")
        nc.vector.memset(ones_r, 1.0)
        eps_t = p_const.tile([1, 1], F32, tag="eps_t")
        nc.vector.memset(eps_t, 1e-5)
        one32 = p_const.tile([128, 1], F32, tag="one32")
        nc.vector.memset(one32, 1.0)

        def act_silu(out, in_, bias=0.0):
            if not sim_compat:
                nc.scalar.activation(out, in_, AF.Silu, bias=bias)
                return
            sg = p_tmp.tile(list(out.shape), F32, tag="simtmp", bufs=2, name="simsg")
            nc.scalar.activation(sg, in_, AF.Sigmoid, bias=bias)
            pre = p_tmp.tile(list(out.shape), F32, tag="simtmp", bufs=2, name="simpre")
            nc.scalar.activation(pre, in_, AF.Identity, bias=bias)
            nc.vector.tensor_tensor(out=out, in0=sg, in1=pre, op=OP.mult)

        def act_softplus(out, in_, bias):
            # softplus = ln(1 + exp(x)); exp and ln share an ACT table set
            e = p_tmp.tile(list(out.shape), F32, tag="spe", bufs=2, name="sime")
            nc.scalar.activation(e, in_, AF.Exp, bias=bias)
            nc.scalar.activation(out, e, AF.Ln, bias=one32)

        # ==================================================================
        # TSSE
        # ==================================================================
        hres = []
        with tc.tile_pool(name="tsse", bufs=2) as p_ts:
            xpz, xpm = [], []
            for c in range(4):
                tz = p_ts.tile([128, R * 9], F16, tag="xp", bufs=8, name=f"xpz{c}")
                nc.sync.dma_start(out=tz, in_=d['xpz'][c])
                xpz.append(tz)
                tm = p_ts.tile([128, R * 9], F16, tag="xp", bufs=8, name=f"xpm{c}")
                nc.sync.dma_start(out=tm, in_=d['xpm'][c])
                xpm.append(tm)

            y_sb = []
            ps_mt = [ps_b.tile([128, T], F32, tag="psb", name=f"psmt{i}")
                     for i in range(2)]
            first = True
            for k in range(3):
                for kc in range(4):
                    wt = p_w.tile([128, 256], F16, tag="wa", name="wa_t")
                    nc.sync.dma_start(out=wt, in_=d['wa'][k, kc])
                    for mt in range(2):
                        for f in range(2):
                            rhs = ap4(xpz[kc], k + f * 128 * 9, [[9, 128], [2, L]])
                            mm(out=ps_mt[mt][:, f * 512:(f + 1) * 512],
                               lhsT=wt[:, mt * 128:(mt + 1) * 128], rhs=rhs,
                               start=first, stop=(k == 2 and kc == 3))
                    first = False
            for mt in range(2):
                a_t = p_ts.tile([128, T], F16, tag="ya", bufs=4, name=f"ya{mt}")
                nc.scalar.activation(a_t, ps_mt[mt], AF.Relu)
                y_sb.append(a_t)

            p_tiles = []
            for c in range(4):
                m1 = p_ts.tile([128, T], F16, tag="mp", bufs=4, name=f"mp{c}")
                nc.vector.tensor_tensor(out=m1, in0=ap4(xpm[c], 0, [[9, R], [2, L]]),
                                        in1=ap4(xpm[c], 1, [[9, R], [2, L]]),
                                        op=OP.max)
                nc.vector.tensor_tensor(out=m1, in0=m1,
                                        in1=ap4(xpm[c], 2, [[9, R], [2, L]]),
                                        op=OP.max)
                p_tiles.append(m1)
            ps_mt = [ps_b.tile([128, T], F32, tag="psb", name=f"psmu{i}")
                     for i in range(2)]
            for kc in range(4):
                wt = p_w.tile([128, 256], F16, tag="wa", name="wp_t")
                nc.sync.dma_start(out=wt, in_=d['wp'][kc])
                for mt in range(2):
                    for f in range(2):
                        mm(out=ps_mt[mt][:, f * 512:(f + 1) * 512],
                           lhsT=wt[:, mt * 128:(mt + 1) * 128],
                           rhs=p_tiles[kc][:, f * 512:(f + 1) * 512],
                           start=(kc == 0), stop=(kc == 3))
            for mt in range(2):
                p_t = p_ts.tile([128, T], F16, tag="ya", bufs=4, name=f"yb{mt}")
                nc.scalar.activation(p_t, ps_mt[mt], AF.Relu)
                y_sb.append(p_t)

            # SE
            ps1 = ps_a.tile([32, R], F32, tag="psa")
            for kc in range(4):
                ym = p_tmp.tile([128, R], F16, tag="tmp", name="ym")
                with nc.allow_low_precision(reason="4-elem mean"):
                    nc.vector.tensor_reduce(
                        out=ym, in_=y_sb[kc].rearrange("p (r t) -> p r t", t=L),
                        axis=mybir.AxisListType.X, op=OP.add)
                wt = p_w.tile([128, 32], F16, tag="se1", name="se1_t")
                nc.sync.dma_start(out=wt, in_=d['se1'][kc])
                mm(out=ps1, lhsT=wt, rhs=ym, start=(kc == 0), stop=(kc == 3))
            s1 = p_tmp.tile([32, R], F16, tag="tmp", name="s1")
            nc.scalar.activation(s1, ps1, AF.Relu, bias=seb1)
            se2_sb = p_w.tile([32, 512], F16, tag="se2", bufs=1, name="se2_t")
            nc.sync.dma_start(out=se2_sb, in_=d['se2'])
            for c in range(4):
                ps2 = ps_a.tile([128, R], F32, tag="psa")
                mm(out=ps2, lhsT=se2_sb[:, c * 128:(c + 1) * 128], rhs=s1,
                   start=True, stop=True)
                sg = p_tmp.tile([128, R], F16, tag="tmp", name="sg")
                nc.scalar.activation(sg, ps2, AF.Sigmoid, bias=seb2[:, c:c + 1])
                hc = p_hres.tile([128, T], F16, tag="hres", name=f"h0_{c}")
                nc.vector.tensor_tensor(out=hc, in0=y_sb[c],
                                        in1=ap4(sg, 0, [[1, R], [0, L]]), op=OP.mult)
                hres.append(hc)

        # ==================================================================
        # LN helper
        # ==================================================================
        def layer_norm(h_chunks, ln_idx):
            psm = [ps_a.tile([1, 512], F32, tag="psa", name=f"psm{i}")
                   for i in range(2)]
            ps2 = [ps_a.tile([1, 512], F32, tag="psa", name=f"pss{i}")
                   for i in range(2)]
            for c in range(4):
                hh = p_tmp.tile([128, T], F16, tag="tmp", name="hh")
                nc.vector.tensor_tensor(out=hh, in0=h_chunks[c], in1=h_chunks[c],
                                        op=OP.mult)
                for f in range(2):
                    mm(out=psm[f], lhsT=ones1,
                       rhs=h_chunks[c][:, f * 512:(f + 1) * 512],
                       start=(c == 0), stop=(c == 3))
                    mm(out=ps2[f], lhsT=ones1, rhs=hh[:, f * 512:(f + 1) * 512],
                       start=(c == 0), stop=(c == 3))
            rstd = p_tmp.tile([1, T], F16, tag="lnfl", bufs=2, name="rstd")
            mrs = p_tmp.tile([1, T], F16, tag="lnfl", bufs=2, name="mrs")
            for f in range(2):
                mean = p_tmp.tile([1, 512], F32, tag="lnfs", bufs=4, name="mean")
                nc.scalar.mul(mean, psm[f], 1.0 / DM)
                ex2 = p_tmp.tile([1, 512], F32, tag="lnfs", bufs=4, name="ex2")
                nc.scalar.mul(ex2, ps2[f], 1.0 / DM)
                var = p_tmp.tile([1, 512], F32, tag="lnfs", bufs=4, name="var")
                nc.vector.tensor_tensor(out=var, in0=mean, in1=mean, op=OP.mult)
                nc.vector.tensor_tensor(out=var, in0=ex2, in1=var, op=OP.subtract)
                sd = p_tmp.tile([1, 512], F32, tag="lnfs", bufs=4, name="sd")
                nc.scalar.activation(sd, var, AF.Sqrt, bias=eps_t)
                rs32 = p_tmp.tile([1, 512], F32, tag="lnfs", bufs=4, name="rs32")
                nc.vector.reciprocal(rs32, sd)
                fsl = slice(f * 512, (f + 1) * 512)
                nc.vector.tensor_copy(rstd[:, fsl], rs32)
                nc.vector.tensor_tensor(out=mrs[:, fsl], in0=mean, in1=rs32,
                                        op=OP.mult)
            rstd_b = ps_b.tile([128, T], F32, tag="psb", name="rstd_b")
            mrs_b = ps_b.tile([128, T], F32, tag="psb", name="mrs_b")
            for f in range(2):
                fsl = slice(f * 512, (f + 1) * 512)
                mm(out=rstd_b[:, fsl], lhsT=ones_r, rhs=rstd[:, fsl],
                   start=True, stop=True)
                mm(out=mrs_b[:, fsl], lhsT=ones_r, rhs=mrs[:, fsl],
                   start=True, stop=True)
            out_chunks = []
            for c in range(4):
                t1 = p_tmp.tile([128, T], F16, tag="tmp", name="lnt1")
                nc.vector.tensor_tensor(out=t1, in0=h_chunks[c], in1=rstd_b,
                                        op=OP.mult)
                nc.vector.tensor_tensor(out=t1, in0=t1, in1=mrs_b, op=OP.subtract)
                t2 = p_tmp.tile([128, T], F16, tag="hn", bufs=5, name="hn_c")
                col = ln_idx * 4 + c
                nc.vector.tensor_scalar(out=t2, in0=t1,
                                        scalar1=lng[:, col:col + 1],
                                        scalar2=lnb[:, col:col + 1],
                                        op0=OP.mult, op1=OP.add)
                out_chunks.append(t2)
            return out_chunks

        # ==================================================================
        # mamba layers
        # ==================================================================
        with tc.tile_pool(name="xi", bufs=2) as p_xi, \
             tc.tile_pool(name="u", bufs=8) as p_u, \
             tc.tile_pool(name="dth", bufs=2) as p_dth, \
             tc.tile_pool(name="grid", bufs=2) as p_grid, \
             tc.tile_pool(name="bc", bufs=2) as p_bc:
            for l in range(NL):
                hn = layer_norm(hres, l)
                # ---- in_proj ----
                for m in range(16):
                    ps = ps_a.tile([128, 512], F32, tag="psa", name="ipp0")
                    ps2 = ps_a.tile([128, 512], F32, tag="psa", name="ipp1")
                    for kc in range(4):
                        wt = p_w.tile([128, 128], F16, tag="w128", bufs=6, name="ipw")
                        nc.sync.dma_start(out=wt, in_=d['inproj'][l, kc, m])
                        mm(out=ps, lhsT=wt, rhs=hn[kc][:, 0:512],
                           start=(kc == 0), stop=(kc == 3))
                        mm(out=ps2, lhsT=wt, rhs=hn[kc][:, 512:1024],
                           start=(kc == 0), stop=(kc == 3))
                    t = p_tmp.tile([128, T], F16, tag="tmp", name="ipo")
                    if m < 8:
                        nc.scalar.activation(t[:, 0:512], ps, AF.Copy)
                        nc.scalar.activation(t[:, 512:1024], ps2, AF.Copy)
                        (nc.scalar if m % 2 else nc.sync).dma_start(
                            out=d['xisp'][l, m], in_=t)
                    else:
                        act_silu(t[:, 0:512], ps)
                        act_silu(t[:, 512:1024], ps2)
                        (nc.scalar if m % 2 else nc.sync).dma_start(
                            out=d['zsp'][l, m - 8], in_=t)

                for dir_ in range(2):
                    ldc = (l * 2 + dir_) * 8
                    # ---- dwconv + silu ----
                    u = []
                    for c in range(8):
                        xic = p_xi.tile([128, T], F16, tag="xi", name="xic")
                        (nc.scalar if c % 2 else nc.sync).dma_start(
                            out=xic, in_=d['xisp'][l, c])
                        uacc = p_tmp.tile([128, T], F16, tag="tmp", name="uacc")
                        ci = (ldc + c) * 4
                        if dir_ == 0:
                            src3 = xic
                        else:
                            src3 = ap4(xic, 3, [[4, R], [-1, L]])
                        nc.vector.tensor_scalar(out=uacc, in0=src3,
                                                scalar1=dwcw[:, ci + 3:ci + 4],
                                                scalar2=None, op0=OP.mult)
                        for k in (2, 1, 0):
                            sh = 3 - k
                            n_t = L - sh
                            o_ap = ap4(uacc, sh, [[4, R], [1, n_t]])
                            if dir_ == 0:
                                i_ap = ap4(xic, 0, [[4, R], [1, n_t]])
                            else:
                                i_ap = ap4(xic, 3, [[4, R], [-1, n_t]])
                            nc.vector.scalar_tensor_tensor(
                                out=o_ap, in0=i_ap,
                                scalar=dwcw[:, ci + k:ci + k + 1],
                                in1=o_ap, op0=OP.mult, op1=OP.add)
                        ut = p_u.tile([128, T], F16, tag="u", name="ut")
                        act_silu(ut, uacc, bias=dwcb[:, ldc + c:ldc + c + 1])
                        u.append(ut)
                    # ---- x_proj ----
                    dbl = ps_b.tile([128, T], F32, tag="psb", name="dbl")
                    for kc in range(8):
                        wt = p_w.tile([128, 64], F16, tag="xpw", name="xpw_t")
                        nc.sync.dma_start(out=wt, in_=d['xpw'][l, dir_, kc])
                        for f in range(2):
                            mm(out=dbl[0:64, f * 512:(f + 1) * 512],
                               lhsT=wt, rhs=u[kc][:, f * 512:(f + 1) * 512],
                               start=(kc == 0), stop=(kc == 7))
                    dtr = p_tmp.tile([32, T], F16, tag="tmp", name="dtr")
                    nc.scalar.activation(dtr, dbl[0:32, :], AF.Copy)
                    bc32 = p_tmp.tile([32, T], F16, tag="tmp", name="bc32")
                    bc32_tm = ap4(bc32, 0, [[1, R], [R, L]])
                    nc.scalar.activation(bc32_tm, dbl[32:64, :], AF.Copy)
                    bsb, csb = bc32[0:16, :], bc32[16:32, :]
                    dtw_sb = p_w.tile([32, 1024], F16, tag="dtw", bufs=2, name="dtw_t")
                    nc.sync.dma_start(out=dtw_sb, in_=d['dtw'][l, dir_])

                    for hf in range(HKW):
                        hsl = slice(hf * TH, (hf + 1) * TH)
                        # B/C flats to DRAM in (s, t, roi) order, then bcast
                        bsrc = bass.AP(tensor=bsb.tensor,
                                       offset=bsb.offset + hf * RH,
                                       ap=[list(bsb.ap[0]), [R, L], [1, RH]])
                        csrc = bass.AP(tensor=csb.tensor,
                                       offset=csb.offset + hf * RH,
                                       ap=[list(csb.ap[0]), [R, L], [1, RH]])
                        bdst = d['bfd'][l, dir_, hf]
                        bdst = bass.AP(tensor=bdst.tensor, offset=bdst.offset,
                                       ap=[[TH, S], [RH, L], [1, RH]])
                        cdst = d['cfd'][l, dir_, hf]
                        cdst = bass.AP(tensor=cdst.tensor, offset=cdst.offset,
                                       ap=[[TH, S], [RH, L], [1, RH]])
                        nc.sync.dma_start(out=bdst, in_=bsrc)
                        nc.scalar.dma_start(out=cdst, in_=csrc)
                        brep = p_bc.tile([128, GRID], F16, tag="bc", name="brep")
                        crep = p_bc.tile([128, GRID], F16, tag="bc", name="crep")
                        qeng = [nc.sync, nc.scalar, nc.sync, nc.scalar]
                        for i in range(4):
                            qeng[i].dma_start(
                                out=brep[i * 32:(i + 1) * 32],
                                in_=dram_bcast(d['bfd'][l, dir_, hf], 32, GRID))
                            qeng[(i + 1) % 4].dma_start(
                                out=crep[i * 32:(i + 1) * 32],
                                in_=dram_bcast(d['cfd'][l, dir_, hf], 32, GRID))

                        for c in range(8):
                            psd = ps_a.tile([128, 512], F32, tag="psa", name="psd")
                            mm(out=psd, lhsT=dtw_sb[:, c * 128:(c + 1) * 128],
                               rhs=dtr[:, hsl], start=True, stop=True)
                            # dt in t-major (t, roi) layout
                            dth = p_dth.tile([128, TH], F16, tag="dt", bufs=2,
                                             name="dth")
                            dth_tm = ap4(dth, 0, [[1, RH], [RH, L]])
                            act_softplus(dth_tm, psd,
                                         bias=dtbb[:, ldc + c:ldc + c + 1])
                            dtu = p_dth.tile([128, TH], F16, tag="dtu", bufs=2,
                                             name="dtu")
                            u_tm = bass.AP(tensor=u[c].tensor,
                                           offset=u[c].offset + hf * TH,
                                           ap=[list(u[c].ap[0]), [1, L], [4, RH]])
                            nc.vector.tensor_tensor(out=dtu, in0=dth, in1=u_tm,
                                                    op=OP.mult)
                            # dA[s, t, roi] via fused exp; t=0 never touched
                            dA = p_grid.tile([128, GRID], F16, tag="dA", name="dA")
                            in_ap = ap4(dth, RH, [[RH, 3], [1, RH]])
                            for s in range(S):
                                o_ap = ap4(dA, s * TH + RH, [[RH, 3], [1, RH]])
                                csa = (ldc + c) * S + s
                                nc.scalar.activation(o_ap, in_ap, AF.Exp,
                                                     scale=app[:, csa:csa + 1])
                            # w = dtu (bcast s) * brep  -> also holds h after
                            wh = p_grid.tile([128, GRID], F16, tag="wh", name="wh")
                            nc.vector.tensor_tensor(
                                out=wh, in0=ap4(dtu, 0, [[0, S], [1, TH]]),
                                in1=brep, op=OP.mult)
                            # unrolled recurrence over t (h_t stored into wh)
                            SRH = S * RH
                            for t in range(1, L):
                                hm = p_dth.tile([128, SRH], F16, tag="hm", bufs=2,
                                                name="hm")
                                nc.vector.tensor_tensor(
                                    out=hm,
                                    in0=ap4(dA, t * RH, [[TH, S], [1, RH]]),
                                    in1=ap4(wh, (t - 1) * RH, [[TH, S], [1, RH]]),
                                    op=OP.mult)
                                nc.vector.tensor_tensor(
                                    out=ap4(wh, t * RH, [[TH, S], [1, RH]]),
                                    in0=ap4(wh, t * RH, [[TH, S], [1, RH]]),
                                    in1=hm, op=OP.add)
                            # yprod + tree per t, assemble yd_raw (roi, t)
                            ydr = p_dth.tile([128, TH], F16, tag="ydr", bufs=2,
                                             name="ydr")
                            for t in range(L):
                                yp = p_dth.tile([128, SRH], F16, tag="hm", bufs=2,
                                                name="yp")
                                nc.gpsimd.tensor_tensor(
                                    out=yp,
                                    in0=ap4(wh, t * RH, [[TH, S], [1, RH]]),
                                    in1=ap4(crep, t * RH, [[TH, S], [1, RH]]),
                                    op=OP.mult)
                                nc.vector.tensor_tensor(
                                    out=yp[:, :SRH // 2], in0=yp[:, :SRH // 2],
                                    in1=yp[:, SRH // 2:], op=OP.add)
                                nc.vector.tensor_tensor(
                                    out=yp[:, :SRH // 4], in0=yp[:, :SRH // 4],
                                    in1=yp[:, SRH // 4:SRH // 2], op=OP.add)
                                nc.vector.tensor_tensor(
                                    out=yp[:, :SRH // 8], in0=yp[:, :SRH // 8],
                                    in1=yp[:, SRH // 8:SRH // 4], op=OP.add)
                                nc.vector.tensor_tensor(
                                    out=ap4(ydr, t, [[4, RH]]),
                                    in0=yp[:, :RH], in1=yp[:, RH:2 * RH],
                                    op=OP.add)
                            # y_dir = ydr(+flip) + u*D -> DRAM accumulate
                            yd = p_dth.tile([128, TH], F16, tag="yd", bufs=3,
                                            name="yd")
                            if dir_ == 0:
                                nc.vector.scalar_tensor_tensor(
                                    out=yd, in0=u[c][:, hsl],
                                    scalar=dpp[:, ldc + c:ldc + c + 1],
                                    in1=ydr, op0=OP.mult, op1=OP.add)
                                nc.gpsimd.dma_start(out=d['ysum'][l, c][:, hsl],
                                                    in_=yd)
                            else:
                                nc.vector.scalar_tensor_tensor(
                                    out=yd,
                                    in0=ap4(u[c], hf * TH + 3, [[4, RH], [-1, L]]),
                                    scalar=dpp[:, ldc + c:ldc + c + 1],
                                    in1=ap4(ydr, 3, [[4, RH], [-1, L]]),
                                    op0=OP.mult, op1=OP.add)
                                nc.gpsimd.dma_start(out=d['ysum'][l, c][:, hsl],
                                                    in_=yd, accum_op=OP.add)

                # ---- gate + out_proj + residual ----
                yg = []
                for c in range(8):
                    sz = p_tmp.tile([128, T], F16, tag="tmp", name="szr")
                    nc.sync.dma_start(out=sz, in_=d['zsp'][l, c])
                    ys = p_tmp.tile([128, T], F16, tag="tmp", name="ysr")
                    nc.sync.dma_start(out=ys, in_=d['ysum'][l, c])
                    t = p_u.tile([128, T], F16, tag="u", name="ygt")
                    nc.vector.tensor_tensor(out=t, in0=ys, in1=sz, op=OP.mult)
                    yg.append(t)
                new_hres = []
                for m in range(4):
                    ps = ps_a.tile([128, 512], F32, tag="psa", name="opp0")
                    ps2 = ps_a.tile([128, 512], F32, tag="psa", name="opp1")
                    for kc in range(8):
                        wt = p_w.tile([128, 128], F16, tag="w128", bufs=6, name="oww")
                        nc.sync.dma_start(out=wt, in_=d['ow'][l, kc, m])
                        mm(out=ps, lhsT=wt, rhs=yg[kc][:, 0:512],
                           start=(kc == 0), stop=(kc == 7))
                        mm(out=ps2, lhsT=wt, rhs=yg[kc][:, 512:1024],
                           start=(kc == 0), stop=(kc == 7))
                    hc = p_hres.tile([128, T], F16, tag="hres", name="hres_n")
                    nc.vector.tensor_tensor(out=hc[:, 0:512], in0=hres[m][:, 0:512],
                                            in1=ps, op=OP.add)
                    nc.vector.tensor_tensor(out=hc[:, 512:1024],
                                            in0=hres[m][:, 512:1024], in1=ps2,
                                            op=OP.add)
                    new_hres.append(hc)
                hres = new_hres

        # ==================================================================
        # out LN + mean pool + MLP
        # ==================================================================
        hn = layer_norm(hres, 2)
        with tc.tile_pool(name="mlp", bufs=6) as p_mlp, \
             tc.tile_pool(name="z1p", bufs=33) as p_z1:
            feat = []
            for c in range(4):
                t = p_mlp.tile([128, R], F16, tag="feat", name="feat_c")
                with nc.allow_low_precision(reason="4-elem mean"):
                    nc.vector.tensor_reduce(
                        out=t, in_=hn[c].rearrange("p (r t) -> p r t", t=L),
                        axis=mybir.AxisListType.X, op=OP.add)
                feat.append(t)
            z1 = []
            for m in range(32):
                ps = ps_a.tile([128, R], F32, tag="psa", name="z1ps")
                for kc in range(4):
                    wt = p_w.tile([128, 128], F16, tag="w128", bufs=6, name="w1t")
                    nc.sync.dma_start(out=wt,
                                      in_=d['w1'][kc, :, m * 128:(m + 1) * 128])
                    mm(out=ps, lhsT=wt, rhs=feat[kc],
                       start=(kc == 0), stop=(kc == 3))
                t = p_z1.tile([128, R], F16, tag="z1", name="z1t")
                nc.scalar.activation(t, ps, AF.Relu, bias=b1c[:, m:m + 1])
                z1.append(t)

            for q in range(4):
                acc_a = [ps_a.tile([128, 512], F32, tag="psa", name=f"acca{i}")
                         for i in range(4)]
                acc_b = [ps_b.tile([128, T], F32, tag="psb", name=f"accb{i}")
                         for i in range(2)]

                def acc_ap(mi):
                    if mi < 4:
                        return acc_a[mi][:, 0:256]
                    j = mi - 4
                    return acc_b[j // 2][:, (j % 2) * 512:(j % 2) * 512 + 256]

                for kc in range(32):
                    slab = p_mlp.tile([128, 1024], F16, tag="w2s", name="slab")
                    (nc.sync if kc % 2 else nc.scalar).dma_start(
                        out=slab, in_=d['w2'][kc, :, q * 1024:(q + 1) * 1024])
                    for mi in range(8):
                        mm(out=acc_ap(mi), lhsT=slab[:, mi * 128:(mi + 1) * 128],
                           rhs=z1[kc], start=(kc == 0), stop=(kc == 31))
                for mi in range(8):
                    mt = q * 8 + mi
                    o_sb = p_mlp.tile([128, R], F32, tag="osb", name="o_sb")
                    nc.scalar.activation(o_sb, acc_ap(mi),
                                         AF.Relu, bias=b2c[:, mt:mt + 1])
                    for rh in range(2):
                        pst = ps_a.tile([128, 128], F32, tag="psa", name="pst")
                        nc.tensor.transpose(pst, o_sb[:, rh * 128:(rh + 1) * 128],
                                            ident)
                        ot = p_mlp.tile([128, 128], F32, tag="ot", name="ot")
                        nc.vector.tensor_copy(ot, pst)
                        nc.sync.dma_start(
                            out=d['out'][rh * 128:(rh + 1) * 128,
                                         mt * 128:(mt + 1) * 128],
                            in_=ot)
    nc.compile()
    return nc


# --------------------------------------------------------------------------
# entry point
# --------------------------------------------------------------------------

def kernel(**inputs):
    if 'nc' not in _COMPILED:
        _COMPILED['nc'] = build_program()
    nc = _COMPILED['nc']
    in_maps = _prep(inputs)
    res = run_bass_kernel_spmd(nc, in_maps, core_ids=list(range(NC)))
    out = np.concatenate([res.results[c]['out'] for c in range(NC)], axis=0)
    return out.astype(np.float32)


if __name__ == '__main__':
    build_program()
    print("program built ok")


# revision 25
# speedup vs baseline: 1.5095x; 1.0499x over previous
"""Trainium2 Bass kernel for nn_ROIHead_TSSEMamba (N=2048 ROIs, 8 cores DP).

Self-contained: host-side packing + Bass/Tile program + SPMD run on 8 cores.

Per-core layout: features on partitions, tokens (roi, t) along free dims.
The selective scan runs on [d_chunk=128, (s=16, roi=128, t=4)] grids with
exp(A*dt) fused into ACT (per-partition scale), one tensor_tensor_scan per
tile, and an in-place tree-add over s.
"""
import numpy as np
from contextlib import ExitStack

import concourse.bass as bass
import concourse.bacc as bacc
import concourse.tile as tile
from concourse import mybir
from concourse.bass_utils import run_bass_kernel_spmd

F16 = mybir.dt.float16
F32 = mybir.dt.float32
AF = mybir.ActivationFunctionType
OP = mybir.AluOpType

NC = 8
NF = 2048
R = NF // NC               # rois per core (256)
L = 4
T = R * L                  # tokens per core (1024)
DM = 512
DI = 1024
S = 16
NL = 2
HID = 4096
HKW = 2                    # roi halves
RH = R // HKW              # rois per half (128)
TH = RH * L                # tokens per half (512)
GRID = S * RH * L          # 8192

_COMPILED = {}


# --------------------------------------------------------------------------
# host-side packing
# --------------------------------------------------------------------------

def _prep(inputs):
    f16 = np.float16
    f32 = np.float32
    g = {}
    wa = np.asarray(inputs['conv_a_w'], f32)            # [256, 512, 3]
    g['wa'] = np.ascontiguousarray(
        wa.transpose(2, 1, 0).reshape(3, 4, 128, 256)).astype(f16)
    wp = np.asarray(inputs['conv_p_w'], f32)[:, :, 0]   # [256, 512]
    g['wp'] = np.ascontiguousarray(wp.T.reshape(4, 128, 256)).astype(f16)
    g['se1'] = np.ascontiguousarray(
        np.asarray(inputs['se_w1'], f32).T.reshape(4, 128, 32) * 0.25).astype(f16)
    g['se2'] = np.ascontiguousarray(np.asarray(inputs['se_w2'], f32).T).astype(f16)
    g['seb1'] = np.asarray(inputs['se_b1'], f32).reshape(32, 1)
    g['seb2'] = np.ascontiguousarray(
        np.asarray(inputs['se_b2'], f32).reshape(4, 128).T)

    lng = np.stack([np.asarray(inputs['ln_g'], f32)[0],
                    np.asarray(inputs['ln_g'], f32)[1],
                    np.asarray(inputs['out_ln_g'], f32)])
    lnb = np.stack([np.asarray(inputs['ln_b'], f32)[0],
                    np.asarray(inputs['ln_b'], f32)[1],
                    np.asarray(inputs['out_ln_b'], f32)])
    g['lng'] = np.ascontiguousarray(
        lng.reshape(3, 4, 128).transpose(2, 0, 1).reshape(128, 12))
    g['lnb'] = np.ascontiguousarray(
        lnb.reshape(3, 4, 128).transpose(2, 0, 1).reshape(128, 12))

    g['inproj'] = np.ascontiguousarray(
        np.asarray(inputs['in_proj_w'], f32).transpose(0, 2, 1)
        .reshape(NL, 4, 128, 16, 128).transpose(0, 1, 3, 2, 4)).astype(f16)

    cw = np.asarray(inputs['conv_w'], f32)              # [2,2,1024,4]
    g['dwcw'] = np.ascontiguousarray(
        cw.reshape(2, 2, 8, 128, 4).transpose(3, 0, 1, 2, 4).reshape(128, 128))
    g['dwcb'] = np.ascontiguousarray(
        np.asarray(inputs['conv_b'], f32).reshape(2, 2, 8, 128)
        .transpose(3, 0, 1, 2).reshape(128, 32))
    g['dtbb'] = np.ascontiguousarray(
        np.asarray(inputs['dt_proj_b'], f32).reshape(2, 2, 8, 128)
        .transpose(3, 0, 1, 2).reshape(128, 32))
    g['dpp'] = np.ascontiguousarray(
        np.asarray(inputs['D_param'], f32).reshape(2, 2, 8, 128)
        .transpose(3, 0, 1, 2).reshape(128, 32))
    A = -np.exp(np.asarray(inputs['A_log'], f32))       # [2,2,1024,16]
    g['app'] = np.ascontiguousarray(
        A.reshape(2, 2, 8, 128, 16).transpose(3, 0, 1, 2, 4).reshape(128, 512))

    g['xpw'] = np.ascontiguousarray(
        np.asarray(inputs['x_proj_w'], f32).transpose(0, 1, 3, 2)
        .reshape(2, 2, 8, 128, 64)).astype(f16)
    g['dtw'] = np.ascontiguousarray(
        np.asarray(inputs['dt_proj_w'], f32).transpose(0, 1, 3, 2)).astype(f16)
    g['ow'] = np.ascontiguousarray(
        np.asarray(inputs['out_proj_w'], f32).transpose(0, 2, 1)
        .reshape(NL, 8, 128, 4, 128).transpose(0, 1, 3, 2, 4)).astype(f16)

    g['w1'] = np.ascontiguousarray(
        np.asarray(inputs['mlp_w1'], f32).T.reshape(4, 128, HID) * 0.25).astype(f16)
    g['b1'] = np.ascontiguousarray(
        np.asarray(inputs['mlp_b1'], f32).reshape(32, 128).T)
    w2t = np.asarray(inputs['mlp_w2'], f32).T.reshape(32, 128, 4, 1024)
    g['w2'] = np.ascontiguousarray(w2t.transpose(2, 0, 1, 3)).astype(f16)
    g['b2'] = np.ascontiguousarray(
        np.asarray(inputs['mlp_b2'], f32).reshape(32, 128).T)
    g['ident'] = np.eye(128, dtype=f32)

    in_maps = []
    xf = np.asarray(inputs['x_flat'], f32)
    for c in range(NC):
        m = dict(g)
        xs = xf[c * R:(c + 1) * R].reshape(R, DM, 7)
        xt = np.transpose(xs, (1, 0, 2))
        xz = np.zeros((DM, R, 9), f32)
        xz[:, :, 1:8] = xt
        xm = np.full((DM, R, 9), -60000.0, f32)
        xm[:, :, 1:8] = xt
        m['xpz'] = np.ascontiguousarray(xz.reshape(4, 128, R * 9)).astype(f16)
        m['xpm'] = np.ascontiguousarray(xm.reshape(4, 128, R * 9)).astype(f16)
        in_maps.append(m)
    return in_maps


# --------------------------------------------------------------------------
# program builder
# --------------------------------------------------------------------------

def _decl(nc):
    d = {}
    def di(name, shape, dt=F16):
        d[name] = nc.dram_tensor(name, shape, dt, kind="ExternalInput").ap()
    di('xpz', [4, 128, R * 9]); di('xpm', [4, 128, R * 9])
    di('wa', [3, 4, 128, 256]); di('wp', [4, 128, 256])
    di('se1', [4, 128, 32]); di('se2', [32, 512])
    di('seb1', [32, 1], F32); di('seb2', [128, 4], F32)
    di('lng', [128, 12], F32); di('lnb', [128, 12], F32)
    di('inproj', [NL, 4, 16, 128, 128])
    di('dwcw', [128, 128], F32); di('dwcb', [128, 32], F32)
    di('dtbb', [128, 32], F32); di('dpp', [128, 32], F32)
    di('app', [128, 512], F32)
    di('xpw', [2, 2, 8, 128, 64]); di('dtw', [2, 2, 32, 1024])
    di('ow', [NL, 8, 4, 128, 128])
    di('w1', [4, 128, HID]); di('b1', [128, 32], F32)
    di('w2', [4, 32, 128, 1024]); di('b2', [128, 32], F32)
    di('ident', [128, 128], F32)
    d['out'] = nc.dram_tensor('out', [R, HID], F32, kind="ExternalOutput").ap()
    d['zsp'] = nc.dram_tensor('zsp', [NL, 8, 128, T], F16).ap()
    d['xisp'] = nc.dram_tensor('xisp', [NL, 8, 128, T], F16).ap()
    d['ysum'] = nc.dram_tensor('ysum', [NL, 8, 128, T], F16).ap()
    d['bfd'] = nc.dram_tensor('bfd', [NL, 2, HKW, GRID], F16).ap()
    d['cfd'] = nc.dram_tensor('cfd', [NL, 2, HKW, GRID], F16).ap()
    return d


def ap4(tile_ap, off, dims):
    """AP with partition dim of tile_ap plus given [step,count] free dims."""
    return bass.AP(tensor=tile_ap.tensor, offset=tile_ap.offset + off,
                   ap=[list(tile_ap.ap[0])] + [list(x) for x in dims])


def dram_bcast(dram_ap, nparts, n):
    """Partition-broadcast read AP of a 1-D DRAM region."""
    return bass.AP(tensor=dram_ap.tensor, offset=dram_ap.offset,
                   ap=[[0, nparts], [1, n]])


def build_program(sim_compat=False):
    nc = bacc.Bacc(debug=False)
    d = _decl(nc)
    ctx = ExitStack()
    with ctx:
        tc = ctx.enter_context(tile.TileContext(nc))
        p_const = ctx.enter_context(tc.tile_pool(name="const", bufs=1))
        p_w = ctx.enter_context(tc.tile_pool(name="wts", bufs=3))
        p_tmp = ctx.enter_context(tc.tile_pool(name="tmp", bufs=6))
        p_hres = ctx.enter_context(tc.tile_pool(name="hres", bufs=5))
        ps_a = ctx.enter_context(tc.tile_pool(name="psa", bufs=4, space="PSUM"))
        ps_b = ctx.enter_context(tc.tile_pool(name="psb", bufs=2, space="PSUM"))

        mm = nc.tensor.matmul

        def cload(name, shape, dt=F32):
            t = p_const.tile(shape, dt, tag=name, name=name + "_c")
            nc.sync.dma_start(out=t, in_=d[name])
            return t
        dwcw = cload('dwcw', [128, 128]); dwcb = cload('dwcb', [128, 32])
        dtbb = cload('dtbb', [128, 32]); dpp = cload('dpp', [128, 32])
        app = cload('app', [128, 512])
        lng = cload('lng', [128, 12]); lnb = cload('lnb', [128, 12])
        seb1 = cload('seb1', [32, 1]); seb2 = cload('seb2', [128, 4])
        b1c = cload('b1', [128, 32]); b2c = cload('b2', [128, 32])
        ident = cload('ident', [128, 128])
        ones1 = p_const.tile([128, 1], F16, tag="ones1")
        nc.vector.memset(ones1, 1.0)
        ones_r = p_const.tile([1, 128], F16, tag="ones_r")
        nc.vector.memset(ones_r, 1.0)
        eps_t = p_const.tile([1, 1], F32, tag="eps_t")
        nc.vector.memset(eps_t, 1e-5)
        one32 = p_const.tile([128, 1], F32, tag="one32")
        nc.vector.memset(one32, 1.0)

        def act_silu(out, in_, bias=0.0):
            if not sim_compat:
                nc.scalar.activation(out, in_, AF.Silu, bias=bias)
                return
            sg = p_tmp.tile(list(out.shape), F32, tag="simtmp", bufs=2, name="simsg")
            nc.scalar.activation(sg, in_, AF.Sigmoid, bias=bias)
            pre = p_tmp.tile(list(out.shape), F32, tag="simtmp", bufs=2, name="simpre")
            nc.scalar.activation(pre, in_, AF.Identity, bias=bias)
            nc.vector.tensor_tensor(out=out, in0=sg, in1=pre, op=OP.mult)

        def act_softplus(out, in_, bias):
            # softplus = ln(1 + exp(x)); exp and ln share an ACT table set
            e = p_tmp.tile(list(out.shape), F32, tag="spe", bufs=2, name="sime")
            nc.scalar.activation(e, in_, AF.Exp, bias=bias)
            nc.scalar.activation(out, e, AF.Ln, bias=one32)

        # ==================================================================
        # TSSE
        # ==================================================================
        hres = []
        with tc.tile_pool(name="tsse", bufs=2) as p_ts:
            xpz, xpm = [], []
            for c in range(4):
                tz = p_ts.tile([128, R * 9], F16, tag="xp", bufs=8, name=f"xpz{c}")
                nc.sync.dma_start(out=tz, in_=d['xpz'][c])
                xpz.append(tz)
                tm = p_ts.tile([128, R * 9], F16, tag="xp", bufs=8, name=f"xpm{c}")
                nc.sync.dma_start(out=tm, in_=d['xpm'][c])
                xpm.append(tm)

            y_sb = []
            ps_mt = [ps_b.tile([128, T], F32, tag="psb", name=f"psmt{i}")
                     for i in range(2)]
            first = True
            for k in range(3):
                for kc in range(4):
                    wt = p_w.tile([128, 256], F16, tag="wa", name="wa_t")
                    nc.sync.dma_start(out=wt, in_=d['wa'][k, kc])
                    for mt in range(2):
                        for f in range(2):
                            rhs = ap4(xpz[kc], k + f * 128 * 9, [[9, 128], [2, L]])
                            mm(out=ps_mt[mt][:, f * 512:(f + 1) * 512],
                               lhsT=wt[:, mt * 128:(mt + 1) * 128], rhs=rhs,
                               start=first, stop=(k == 2 and kc == 3))
                    first = False
            for mt in range(2):
                a_t = p_ts.tile([128, T], F16, tag="ya", bufs=4, name=f"ya{mt}")
                nc.scalar.activation(a_t, ps_mt[mt], AF.Relu)
                y_sb.append(a_t)

            p_tiles = []
            for c in range(4):
                m1 = p_ts.tile([128, T], F16, tag="mp", bufs=4, name=f"mp{c}")
                nc.vector.tensor_tensor(out=m1, in0=ap4(xpm[c], 0, [[9, R], [2, L]]),
                                        in1=ap4(xpm[c], 1, [[9, R], [2, L]]),
                                        op=OP.max)
                nc.vector.tensor_tensor(out=m1, in0=m1,
                                        in1=ap4(xpm[c], 2, [[9, R], [2, L]]),
                                        op=OP.max)
                p_tiles.append(m1)
            ps_mt = [ps_b.tile([128, T], F32, tag="psb", name=f"psmu{i}")
                     for i in range(2)]
            for kc in range(4):
                wt = p_w.tile([128, 256], F16, tag="wa", name="wp_t")
                nc.sync.dma_start(out=wt, in_=d['wp'][kc])
                for mt in range(2):
                    for f in range(2):
                        mm(out=ps_mt[mt][:, f * 512:(f + 1) * 512],
                           lhsT=wt[:, mt * 128:(mt + 1) * 128],
                           rhs=p_tiles[kc][:, f * 512:(f + 1) * 512],
                           start=(kc == 0), stop=(kc == 3))
            for mt in range(2):
                p_t = p_ts.tile([128, T], F16, tag="ya", bufs=4, name=f"yb{mt}")
                nc.scalar.activation(p_t, ps_mt[mt], AF.Relu)
                y_sb.append(p_t)

            # SE
            ps1 = ps_a.tile([32, R], F32, tag="psa")
            for kc in range(4):
                ym = p_tmp.tile([128, R], F16, tag="tmp", name="ym")
                with nc.allow_low_precision(reason="4-elem mean"):
                    nc.vector.tensor_reduce(
                        out=ym, in_=y_sb[kc].rearrange("p (r t) -> p r t", t=L),
                        axis=mybir.AxisListType.X, op=OP.add)
                wt = p_w.tile([128, 32], F16, tag="se1", name="se1_t")
                nc.sync.dma_start(out=wt, in_=d['se1'][kc])
                mm(out=ps1, lhsT=wt, rhs=ym, start=(kc == 0), stop=(kc == 3))
            s1 = p_tmp.tile([32, R], F16, tag="tmp", name="s1")
            nc.scalar.activation(s1, ps1, AF.Relu, bias=seb1)
            se2_sb = p_w.tile([32, 512], F16, tag="se2", bufs=1, name="se2_t")
            nc.sync.dma_start(out=se2_sb, in_=d['se2'])
            for c in range(4):
                ps2 = ps_a.tile([128, R], F32, tag="psa")
                mm(out=ps2, lhsT=se2_sb[:, c * 128:(c + 1) * 128], rhs=s1,
                   start=True, stop=True)
                sg = p_tmp.tile([128, R], F16, tag="tmp", name="sg")
                nc.scalar.activation(sg, ps2, AF.Sigmoid, bias=seb2[:, c:c + 1])
                hc = p_hres.tile([128, T], F16, tag="hres", name=f"h0_{c}")
                nc.vector.tensor_tensor(out=hc, in0=y_sb[c],
                                        in1=ap4(sg, 0, [[1, R], [0, L]]), op=OP.mult)
                hres.append(hc)

        # ==================================================================
        # LN helper
        # ==================================================================
        def layer_norm(h_chunks, ln_idx):
            psm = [ps_a.tile([1, 512], F32, tag="psa", name=f"psm{i}")
                   for i in range(2)]
            ps2 = [ps_a.tile([1, 512], F32, tag="psa", name=f"pss{i}")
                   for i in range(2)]
            for c in range(4):
                hh = p_tmp.tile([128, T], F16, tag="tmp", name="hh")
                nc.vector.tensor_tensor(out=hh, in0=h_chunks[c], in1=h_chunks[c],
                                        op=OP.mult)
                for f in range(2):
                    mm(out=psm[f], lhsT=ones1,
                       rhs=h_chunks[c][:, f * 512:(f + 1) * 512],
                       start=(c == 0), stop=(c == 3))
                    mm(out=ps2[f], lhsT=ones1, rhs=hh[:, f * 512:(f + 1) * 512],
                       start=(c == 0), stop=(c == 3))
            rstd = p_tmp.tile([1, T], F16, tag="lnfl", bufs=2, name="rstd")
            mrs = p_tmp.tile([1, T], F16, tag="lnfl", bufs=2, name="mrs")
            for f in range(2):
                mean = p_tmp.tile([1, 512], F32, tag="lnfs", bufs=4, name="mean")
                nc.scalar.mul(mean, psm[f], 1.0 / DM)
                ex2 = p_tmp.tile([1, 512], F32, tag="lnfs", bufs=4, name="ex2")
                nc.scalar.mul(ex2, ps2[f], 1.0 / DM)
                var = p_tmp.tile([1, 512], F32, tag="lnfs", bufs=4, name="var")
                nc.vector.tensor_tensor(out=var, in0=mean, in1=mean, op=OP.mult)
                nc.vector.tensor_tensor(out=var, in0=ex2, in1=var, op=OP.subtract)
                sd = p_tmp.tile([1, 512], F32, tag="lnfs", bufs=4, name="sd")
                nc.scalar.activation(sd, var, AF.Sqrt, bias=eps_t)
                rs32 = p_tmp.tile([1, 512], F32, tag="lnfs", bufs=4, name="rs32")
                nc.vector.reciprocal(rs32, sd)
                fsl = slice(f * 512, (f + 1) * 512)
                nc.vector.tensor_copy(rstd[:, fsl], rs32)
                nc.vector.tensor_tensor(out=mrs[:, fsl], in0=mean, in1=rs32,
                                        op=OP.mult)
            rstd_b = ps_b.tile([128, T], F32, tag="psb", name="rstd_b")
            mrs_b = ps_b.tile([128, T], F32, tag="psb", name="mrs_b")
            for f in range(2):
                fsl = slice(f * 512, (f + 1) * 512)
                mm(out=rstd_b[:, fsl], lhsT=ones_r, rhs=rstd[:, fsl],
                   start=True, stop=True)
                mm(out=mrs_b[:, fsl], lhsT=ones_r, rhs=mrs[:, fsl],
                   start=True, stop=True)
            out_chunks = []
            for c in range(4):
                t1 = p_tmp.tile([128, T], F16, tag="tmp", name="lnt1")
                nc.vector.tensor_tensor(out=t1, in0=h_chunks[c], in1=rstd_b,
                                        op=OP.mult)
                nc.vector.tensor_tensor(out=t1, in0=t1, in1=mrs_b, op=OP.subtract)
                t2 = p_tmp.tile([128, T], F16, tag="hn", bufs=4, name="hn_c")
                col = ln_idx * 4 + c
                nc.vector.tensor_scalar(out=t2, in0=t1,
                                        scalar1=lng[:, col:col + 1],
                                        scalar2=lnb[:, col:col + 1],
                                        op0=OP.mult, op1=OP.add)
                out_chunks.append(t2)
            return out_chunks

        # ==================================================================
        # mamba layers
        # ==================================================================
        with tc.tile_pool(name="xi", bufs=2) as p_xi, \
             tc.tile_pool(name="u", bufs=8) as p_u, \
             tc.tile_pool(name="dth", bufs=2) as p_dth, \
             tc.tile_pool(name="grid", bufs=2) as p_grid, \
             tc.tile_pool(name="bc", bufs=3) as p_bc:
            for l in range(NL):
                hn = layer_norm(hres, l)
                # ---- in_proj ----
                for m in range(16):
                    ps = ps_a.tile([128, 512], F32, tag="psa", name="ipp0")
                    ps2 = ps_a.tile([128, 512], F32, tag="psa", name="ipp1")
                    for kc in range(4):
                        wt = p_w.tile([128, 128], F16, tag="w128", bufs=6, name="ipw")
                        nc.sync.dma_start(out=wt, in_=d['inproj'][l, kc, m])
                        mm(out=ps, lhsT=wt, rhs=hn[kc][:, 0:512],
                           start=(kc == 0), stop=(kc == 3))
                        mm(out=ps2, lhsT=wt, rhs=hn[kc][:, 512:1024],
                           start=(kc == 0), stop=(kc == 3))
                    t = p_tmp.tile([128, T], F16, tag="tmp", name="ipo")
                    if m < 8:
                        nc.scalar.activation(t[:, 0:512], ps, AF.Copy)
                        nc.scalar.activation(t[:, 512:1024], ps2, AF.Copy)
                        (nc.scalar if m % 2 else nc.sync).dma_start(
                            out=d['xisp'][l, m], in_=t)
                    else:
                        act_silu(t[:, 0:512], ps)
                        act_silu(t[:, 512:1024], ps2)
                        (nc.scalar if m % 2 else nc.sync).dma_start(
                            out=d['zsp'][l, m - 8], in_=t)

                for dir_ in range(2):
                    ldc = (l * 2 + dir_) * 8
                    # ---- dwconv + silu ----
                    u = []
                    for c in range(8):
                        xic = p_xi.tile([128, T], F16, tag="xi", name="xic")
                        (nc.scalar if c % 2 else nc.sync).dma_start(
                            out=xic, in_=d['xisp'][l, c])
                        uacc = p_tmp.tile([128, T], F16, tag="tmp", name="uacc")
                        ci = (ldc + c) * 4
                        if dir_ == 0:
                            src3 = xic
                        else:
                            src3 = ap4(xic, 3, [[4, R], [-1, L]])
                        nc.vector.tensor_scalar(out=uacc, in0=src3,
                                                scalar1=dwcw[:, ci + 3:ci + 4],
                                                scalar2=None, op0=OP.mult)
                        for k in (2, 1, 0):
                            sh = 3 - k
                            n_t = L - sh
                            o_ap = ap4(uacc, sh, [[4, R], [1, n_t]])
                            if dir_ == 0:
                                i_ap = ap4(xic, 0, [[4, R], [1, n_t]])
                            else:
                                i_ap = ap4(xic, 3, [[4, R], [-1, n_t]])
                            nc.vector.scalar_tensor_tensor(
                                out=o_ap, in0=i_ap,
                                scalar=dwcw[:, ci + k:ci + k + 1],
                                in1=o_ap, op0=OP.mult, op1=OP.add)
                        ut = p_u.tile([128, T], F16, tag="u", name="ut")
                        act_silu(ut, uacc, bias=dwcb[:, ldc + c:ldc + c + 1])
                        u.append(ut)
                    # ---- x_proj ----
                    dbl = ps_b.tile([128, T], F32, tag="psb", name="dbl")
                    for kc in range(8):
                        wt = p_w.tile([128, 64], F16, tag="xpw", name="xpw_t")
                        nc.sync.dma_start(out=wt, in_=d['xpw'][l, dir_, kc])
                        for f in range(2):
                            mm(out=dbl[0:64, f * 512:(f + 1) * 512],
                               lhsT=wt, rhs=u[kc][:, f * 512:(f + 1) * 512],
                               start=(kc == 0), stop=(kc == 7))
                    dtr = p_tmp.tile([32, T], F16, tag="tmp", name="dtr")
                    nc.scalar.activation(dtr, dbl[0:32, :], AF.Copy)
                    bc32 = p_tmp.tile([32, T], F16, tag="tmp", name="bc32")
                    bc32_tm = ap4(bc32, 0, [[1, R], [R, L]])
                    nc.scalar.activation(bc32_tm, dbl[32:64, :], AF.Copy)
                    bsb, csb = bc32[0:16, :], bc32[16:32, :]
                    dtw_sb = p_w.tile([32, 1024], F16, tag="dtw", bufs=2, name="dtw_t")
                    nc.sync.dma_start(out=dtw_sb, in_=d['dtw'][l, dir_])

                    for hf in range(HKW):
                        hsl = slice(hf * TH, (hf + 1) * TH)
                        # B/C flats to DRAM in (s, t, roi) order, then bcast
                        bsrc = bass.AP(tensor=bsb.tensor,
                                       offset=bsb.offset + hf * RH,
                                       ap=[list(bsb.ap[0]), [R, L], [1, RH]])
                        csrc = bass.AP(tensor=csb.tensor,
                                       offset=csb.offset + hf * RH,
                                       ap=[list(csb.ap[0]), [R, L], [1, RH]])
                        bdst = d['bfd'][l, dir_, hf]
                        bdst = bass.AP(tensor=bdst.tensor, offset=bdst.offset,
                                       ap=[[TH, S], [RH, L], [1, RH]])
                        cdst = d['cfd'][l, dir_, hf]
                        cdst = bass.AP(tensor=cdst.tensor, offset=cdst.offset,
                                       ap=[[TH, S], [RH, L], [1, RH]])
                        nc.sync.dma_start(out=bdst, in_=bsrc)
                        nc.scalar.dma_start(out=cdst, in_=csrc)
                        brep = p_bc.tile([128, GRID], F16, tag="bc", name="brep")
                        crep = p_bc.tile([128, GRID], F16, tag="bc", name="crep")
                        qeng = [nc.sync, nc.scalar, nc.gpsimd]
                        for i in range(4):
                            qeng[i % 3].dma_start(
                                out=brep[i * 32:(i + 1) * 32],
                                in_=dram_bcast(d['bfd'][l, dir_, hf], 32, GRID))
                            qeng[(i + 1) % 3].dma_start(
                                out=crep[i * 32:(i + 1) * 32],
                                in_=dram_bcast(d['cfd'][l, dir_, hf], 32, GRID))

                        for c in range(8):
                            psd = ps_a.tile([128, 512], F32, tag="psa", name="psd")
                            mm(out=psd, lhsT=dtw_sb[:, c * 128:(c + 1) * 128],
                               rhs=dtr[:, hsl], start=True, stop=True)
                            # dt in t-major (t, roi) layout
                            dth = p_dth.tile([128, TH], F16, tag="dt", bufs=2,
                                             name="dth")
                            dth_tm = ap4(dth, 0, [[1, RH], [RH, L]])
                            act_softplus(dth_tm, psd,
                                         bias=dtbb[:, ldc + c:ldc + c + 1])
                            # u half in t-major (ACT copy), then dtu on DVE 2x
                            u_tm = p_dth.tile([128, TH], F16, tag="utm", bufs=2,
                                              name="u_tm")
                            u_tv = bass.AP(tensor=u[c].tensor,
                                           offset=u[c].offset + hf * TH,
                                           ap=[list(u[c].ap[0]), [1, L], [4, RH]])
                            nc.scalar.activation(u_tm, u_tv, AF.Copy)
                            dtu = p_dth.tile([128, TH], F16, tag="dtu", bufs=2,
                                             name="dtu")
                            nc.vector.tensor_tensor(out=dtu, in0=dth, in1=u_tm,
                                                    op=OP.mult)
                            # dA[s, t(1..3), roi] via fused exp
                            T3 = 3 * RH
                            dA = p_grid.tile([128, S * T3], F16, tag="dA",
                                             name="dA")
                            in_ap = ap4(dth, RH, [[RH, 3], [1, RH]])
                            for s in range(S):
                                o_ap = ap4(dA, s * T3, [[RH, 3], [1, RH]])
                                csa = (ldc + c) * S + s
                                nc.scalar.activation(o_ap, in_ap, AF.Exp,
                                                     scale=app[:, csa:csa + 1])
                            # w then h (in place), yp (in place), tree (in place)
                            wh = p_grid.tile([128, GRID], F16, tag="wh", name="wh")
                            nc.vector.tensor_tensor(
                                out=wh, in0=ap4(dtu, 0, [[0, S], [1, TH]]),
                                in1=brep, op=OP.mult)
                            for t in range(1, L):
                                hm = p_dth.tile([128, S * RH], F16, tag="hm",
                                                bufs=2, name="hm")
                                nc.vector.tensor_tensor(
                                    out=hm,
                                    in0=ap4(dA, (t - 1) * RH, [[T3, S], [1, RH]]),
                                    in1=ap4(wh, (t - 1) * RH, [[TH, S], [1, RH]]),
                                    op=OP.mult)
                                nc.vector.tensor_tensor(
                                    out=ap4(wh, t * RH, [[TH, S], [1, RH]]),
                                    in0=ap4(wh, t * RH, [[TH, S], [1, RH]]),
                                    in1=hm, op=OP.add)
                            nc.gpsimd.tensor_tensor(out=wh, in0=wh, in1=crep,
                                                    op=OP.mult)
                            nc.vector.tensor_tensor(out=wh[:, :GRID // 2],
                                                    in0=wh[:, :GRID // 2],
                                                    in1=wh[:, GRID // 2:],
                                                    op=OP.add)
                            nc.vector.tensor_tensor(out=wh[:, :GRID // 4],
                                                    in0=wh[:, :GRID // 4],
                                                    in1=wh[:, GRID // 4:GRID // 2],
                                                    op=OP.add)
                            nc.vector.tensor_tensor(out=wh[:, :GRID // 8],
                                                    in0=wh[:, :GRID // 8],
                                                    in1=wh[:, GRID // 8:GRID // 4],
                                                    op=OP.add)
                            nc.vector.tensor_tensor(out=wh[:, :TH],
                                                    in0=wh[:, :TH],
                                                    in1=wh[:, TH:GRID // 8],
                                                    op=OP.add)
                            # y now in wh[:, :TH], (t, roi)-major
                            yd = p_dth.tile([128, TH], F16, tag="yd", bufs=3,
                                            name="yd")
                            if dir_ == 0:
                                yv = ap4(wh, 0, [[1, RH], [RH, L]])
                                nc.vector.scalar_tensor_tensor(
                                    out=yd, in0=u[c][:, hsl],
                                    scalar=dpp[:, ldc + c:ldc + c + 1],
                                    in1=yv, op0=OP.mult, op1=OP.add)
                                nc.gpsimd.dma_start(out=d['ysum'][l, c][:, hsl],
                                                    in_=yd)
                            else:
                                yv = ap4(wh, 3 * RH, [[1, RH], [-RH, L]])
                                nc.vector.scalar_tensor_tensor(
                                    out=yd,
                                    in0=ap4(u[c], hf * TH + 3, [[4, RH], [-1, L]]),
                                    scalar=dpp[:, ldc + c:ldc + c + 1],
                                    in1=yv, op0=OP.mult, op1=OP.add)
                                nc.gpsimd.dma_start(out=d['ysum'][l, c][:, hsl],
                                                    in_=yd, accum_op=OP.add)

                # ---- gate + out_proj + residual ----
                yg = []
                for c in range(8):
                    sz = p_tmp.tile([128, T], F16, tag="tmp", name="szr")
                    nc.sync.dma_start(out=sz, in_=d['zsp'][l, c])
                    ys = p_tmp.tile([128, T], F16, tag="tmp", name="ysr")
                    nc.sync.dma_start(out=ys, in_=d['ysum'][l, c])
                    t = p_u.tile([128, T], F16, tag="u", name="ygt")
                    nc.vector.tensor_tensor(out=t, in0=ys, in1=sz, op=OP.mult)
                    yg.append(t)
                new_hres = []
                for m in range(4):
                    ps = ps_a.tile([128, 512], F32, tag="psa", name="opp0")
                    ps2 = ps_a.tile([128, 512], F32, tag="psa", name="opp1")
                    for kc in range(8):
                        wt = p_w.tile([128, 128], F16, tag="w128", bufs=6, name="oww")
                        nc.sync.dma_start(out=wt, in_=d['ow'][l, kc, m])
                        mm(out=ps, lhsT=wt, rhs=yg[kc][:, 0:512],
                           start=(kc == 0), stop=(kc == 7))
                        mm(out=ps2, lhsT=wt, rhs=yg[kc][:, 512:1024],
                           start=(kc == 0), stop=(kc == 7))
                    hc = p_hres.tile([128, T], F16, tag="hres", name="hres_n")
                    nc.vector.tensor_tensor(out=hc[:, 0:512], in0=hres[m][:, 0:512],
                                            in1=ps, op=OP.add)
                    nc.vector.tensor_tensor(out=hc[:, 512:1024],
                                            in0=hres[m][:, 512:1024], in1=ps2,
                                            op=OP.add)
                    new_hres.append(hc)
                hres = new_hres

        # ==================================================================
        # out LN + mean pool + MLP
        # ==================================================================
        hn = layer_norm(hres, 2)
        with tc.tile_pool(name="mlp", bufs=6) as p_mlp, \
             tc.tile_pool(name="z1p", bufs=33) as p_z1:
            feat = []
            for c in range(4):
                t = p_mlp.tile([128, R], F16, tag="feat", name="feat_c")
                with nc.allow_low_precision(reason="4-elem mean"):
                    nc.vector.tensor_reduce(
                        out=t, in_=hn[c].rearrange("p (r t) -> p r t", t=L),
                        axis=mybir.AxisListType.X, op=OP.add)
                feat.append(t)
            z1 = []
            for m in range(32):
                ps = ps_a.tile([128, R], F32, tag="psa", name="z1ps")
                for kc in range(4):
                    wt = p_w.tile([128, 128], F16, tag="w128", bufs=6, name="w1t")
                    nc.sync.dma_start(out=wt,
                                      in_=d['w1'][kc, :, m * 128:(m + 1) * 128])
                    mm(out=ps, lhsT=wt, rhs=feat[kc],
                       start=(kc == 0), stop=(kc == 3))
                t = p_z1.tile([128, R], F16, tag="z1", name="z1t")
                nc.scalar.activation(t, ps, AF.Relu, bias=b1c[:, m:m + 1])
                z1.append(t)

            for q in range(4):
                acc_a = [ps_a.tile([128, 512], F32, tag="psa", name=f"acca{i}")
                         for i in range(4)]
                acc_b = [ps_b.tile([128, T], F32, tag="psb", name=f"accb{i}")
                         for i in range(2)]

                def acc_ap(mi):
                    if mi < 4:
                        return acc_a[mi][:, 0:256]
                    j = mi - 4
                    return acc_b[j // 2][:, (j % 2) * 512:(j % 2) * 512 + 256]

                for kc in range(32):
                    slab = p_mlp.tile([128, 1024], F16, tag="w2s", name="slab")
                    (nc.sync if kc % 2 else nc.scalar).dma_start(
                        out=slab, in_=d['w2'][q, kc])
                    for mi in range(8):
                        mm(out=acc_ap(mi), lhsT=slab[:, mi * 128:(mi + 1) * 128],
                           rhs=z1[kc], start=(kc == 0), stop=(kc == 31))
                for mi in range(8):
                    mt = q * 8 + mi
                    o_sb = p_mlp.tile([128, R], F32, tag="osb", name="o_sb")
                    nc.scalar.activation(o_sb, acc_ap(mi),
                                         AF.Relu, bias=b2c[:, mt:mt + 1])
                    for rh in range(2):
                        pst = ps_a.tile([128, 128], F32, tag="psa", name="pst")
                        nc.tensor.transpose(pst, o_sb[:, rh * 128:(rh + 1) * 128],
                                            ident)
                        ot = p_mlp.tile([128, 128], F32, tag="ot", name="ot")
                        nc.vector.tensor_copy(ot, pst)
                        nc.sync.dma_start(
                            out=d['out'][rh * 128:(rh + 1) * 128,
                                         mt * 128:(mt + 1) * 128],
                            in_=ot)
    nc.compile()
    return nc


# --------------------------------------------------------------------------
# entry point
# --------------------------------------------------------------------------

def kernel(**inputs):
    if 'nc' not in _COMPILED:
        _COMPILED['nc'] = build_program()
    nc = _COMPILED['nc']
    in_maps = _prep(inputs)
    res = run_bass_kernel_spmd(nc, in_maps, core_ids=list(range(NC)))
    out = np.concatenate([res.results[c]['out'] for c in range(NC)], axis=0)
    return out.astype(np.float32)


if __name__ == '__main__':
    build_program()
    print("program built ok")
